# revision 14
# baseline (speedup 1.0000x reference)
"""Trainium2 Bass kernel for nn_MultiHeadAttention_spatial_shared.

Math (per frame f of BS*SQ=131072, all independent):
  qh[h,n] = sum_d Wq[h,d]*q[f, d*15+n] + bq[h]   (same for k, v)
  aq = |qh|, ak = |kh|
  E[h,n,m] = exp(aq[h,n]*ak[h,m])                (flattened softmax numerator)
  Z[h]     = sum_{n,m} E[h,n,m]
  att[h,n] = sum_m E[h,n,m]*vh[h,m]/Z[h] + 2*sum_m dist[n,m]*vh[h,m]
  out[e]   = sum_{h,n} Wo[e, n*3+h]*att[h,n] + bo[e]

Sharding: pure data parallel over frames across 8 cores (16384 frames/core).

On-chip layout: frames on partitions, 4 frames per partition ("FR"), 512
frames per super-tile, 32 super-tiles per core.  The m axis is padded to 16
(pad of ak is -1e30 so exp()=0; pad of v/dist is 0) so the m-reduction can be
done as a bf16 pairwise tree at 2x DVE rate.  The 45x45 output projection and
its transposes run on the TensorEngine.
"""

import numpy as np

import concourse.bass as bass
import concourse.bacc as bacc
import concourse.tile as tile
from concourse import mybir
from concourse.bass_utils import run_bass_kernel_spmd
from concourse.masks import make_identity

F32 = mybir.dt.float32
BF16 = mybir.dt.bfloat16
U32 = mybir.dt.uint32
AX = mybir.AxisListType
OP = mybir.AluOpType

N_CORES = 8
BS, SQ, DIM, N = 32, 4096, 3, 15
FEAT = DIM * N            # 45
M16 = 16                  # m padded to 16
P = 128                   # partitions
FR = 4                    # frames per partition per super-tile
TILE_F = P * FR           # 512 frames per super-tile
F_CORE = BS * SQ // N_CORES  # 16384


def build_nc(n_tiles=F_CORE // TILE_F):
    f_core = n_tiles * TILE_F
    nc = bacc.Bacc("TRN2", target_bir_lowering=False, debug=False)

    qx = nc.dram_tensor("q", [f_core, FEAT], F32, kind="ExternalInput").ap()
    kx = nc.dram_tensor("k", [f_core, FEAT], F32, kind="ExternalInput").ap()
    vx = nc.dram_tensor("v", [f_core, FEAT], F32, kind="ExternalInput").ap()
    dx = nc.dram_tensor("d", [f_core, N * N], F32, kind="ExternalInput").ap()
    wcx = nc.dram_tensor("wc", [3 * DIM * DIM], F32, kind="ExternalInput").ap()
    bcx = nc.dram_tensor("bc", [3 * DIM], F32, kind="ExternalInput").ap()
    mox = nc.dram_tensor("mo", [2 * FEAT, 2 * FEAT], F32, kind="ExternalInput").ap()
    box = nc.dram_tensor("bo", [2 * FEAT], F32, kind="ExternalInput").ap()
    ox = nc.dram_tensor("out", [f_core, FEAT], F32, kind="ExternalOutput").ap()

    # DRAM views: super-tile t, partition p, frame-in-partition f
    qv = qx.rearrange("(t p f) e -> t p (f e)", p=P, f=FR)
    kv = kx.rearrange("(t p f) e -> t p (f e)", p=P, f=FR)
    vv = vx.rearrange("(t p f) e -> t p (f e)", p=P, f=FR)
    dv = dx.rearrange("(t p f) e -> t p (f e)", p=P, f=FR)
    ov = ox.rearrange("(t p f) e -> t p (f e)", p=P, f=FR)

    with tile.TileContext(nc) as tc:
        _kernel_body(tc, n_tiles, qv, kv, vv, dv, ov, wcx, bcx, mox, box)
    nc.compile()
    return nc


def _kernel_body(tc, n_tiles, qv, kv, vv, dv, ov, wcx, bcx, mox, box):
    nc = tc.nc
    from contextlib import ExitStack
    with ExitStack() as ctx:
        consts = ctx.enter_context(tc.tile_pool(name="consts", bufs=1))
        dma_in = ctx.enter_context(tc.tile_pool(name="dma_in", bufs=3))
        work = ctx.enter_context(tc.tile_pool(name="work", bufs=2))
        big = ctx.enter_context(tc.tile_pool(name="big", bufs=2))
        outp = ctx.enter_context(tc.tile_pool(name="outp", bufs=3))
        psum = ctx.enter_context(tc.tile_pool(name="psum", bufs=2, space="PSUM"))

        def pbcast(ap, p=P):
            return bass.AP(tensor=ap.tensor, offset=ap.offset,
                           ap=[[0, p]] + list(ap.ap))

        # ---- constants (once) ----
        # DMA into staging tiles, then copy via DVE so downstream compute ops
        # only ever carry a single sync-wait class (walrus TT encoding allows
        # one wait per instruction).
        wc0 = consts.tile([P, 3, DIM, DIM], F32)       # (t, h, d)
        nc.sync.dma_start(out=wc0.rearrange("p a b c -> p (a b c)"),
                          in_=pbcast(wcx))
        wc = consts.tile([P, 3, DIM, DIM], F32)
        nc.vector.tensor_copy(wc, wc0)
        bc0 = consts.tile([P, 3, DIM], F32)            # (t, h)
        nc.sync.dma_start(out=bc0.rearrange("p a b -> p (a b)"),
                          in_=pbcast(bcx))
        bc = consts.tile([P, 3, DIM], F32)
        nc.vector.tensor_copy(bc, bc0)
        mo0 = consts.tile([2 * FEAT, 2 * FEAT], F32)   # blockdiag lhsT [(j,h,n),(j,e)]
        nc.sync.dma_start(out=mo0, in_=mox)
        mo = consts.tile([2 * FEAT, 2 * FEAT], F32)
        nc.vector.tensor_copy(mo, mo0)
        bo20 = consts.tile([2 * FEAT, 1], F32)
        nc.sync.dma_start(out=bo20, in_=box.rearrange("(p one) -> p one", one=1))
        bo2 = consts.tile([2 * FEAT, 1], F32)
        nc.vector.tensor_copy(bo2, bo20)
        idn0 = consts.tile([P, P], F32)
        make_identity(nc, idn0)
        idn = consts.tile([P, P], F32)
        nc.vector.tensor_copy(idn, idn0)

        for it in range(n_tiles):
            # ---- DMA in ----
            qt = dma_in.tile([P, FR, DIM, N], F32, tag="qt")
            kt = dma_in.tile([P, FR, DIM, N], F32, tag="kt")
            vt = dma_in.tile([P, FR, DIM, N], F32, tag="vt")
            dt = dma_in.tile([P, FR, N, N], F32, tag="dt")
            nc.sync.dma_start(out=qt.rearrange("p f d n -> p (f d n)"), in_=qv[it])
            nc.sync.dma_start(out=kt.rearrange("p f d n -> p (f d n)"), in_=kv[it])
            nc.sync.dma_start(out=vt.rearrange("p f d n -> p (f d n)"), in_=vv[it])
            nc.sync.dma_start(out=dt.rearrange("p f n m -> p (f n m)"), in_=dv[it])

            # ---- projections: qkvh[(t, fr, h, m16)] ----
            qkvh = work.tile([P, 3, FR, DIM, M16], F32, tag="qkvh")
            for t, src in enumerate((qt, kt, vt)):
                tmp = work.tile([P, FR, DIM, DIM, N], F32, tag="ptmp")
                nc.vector.tensor_tensor(
                    out=tmp,
                    in0=src[:, :, None, :, :].broadcast_to((P, FR, DIM, DIM, N)),
                    in1=wc[:, t][:, None, :, :, None].broadcast_to((P, FR, DIM, DIM, N)),
                    op=OP.mult)
                nc.vector.reduce_sum(
                    out=qkvh[:, t, :, :, 0:N],
                    in_=tmp.rearrange("p f h d n -> p f h n d"),
                    axis=AX.X)
            # bias (per-tensor: keeps APs within the 3-free-dim ISA limit)
            for t in range(3):
                nc.vector.tensor_tensor(
                    out=qkvh[:, t, :, :, 0:N],
                    in0=qkvh[:, t, :, :, 0:N],
                    in1=bc[:, t][:, None, :, None].broadcast_to((P, FR, DIM, N)),
                    op=OP.add)
            # abs of qh, kh on the Scalar engine (only the 15 real columns)
            nc.scalar.activation(
                out=qkvh[:, 0:2, :, :, 0:N], in_=qkvh[:, 0:2, :, :, 0:N],
                func=mybir.ActivationFunctionType.Abs)
            # pads: k pad -> -1e30 (exp -> 0); v pad -> 0; q pad never read
            nc.vector.memset(qkvh[:, 1, :, :, N:M16], -1e30)
            nc.vector.memset(qkvh[:, 2, :, :, N:M16], 0.0)

            # ---- E_pre = aq (x) ak16,  E = exp(E_pre) in bf16 ----
            e_pre = big.tile([P, FR, DIM, N, M16], F32, tag="e_pre")
            for h in range(DIM):
                nc.vector.tensor_tensor(
                    out=e_pre[:, :, h],
                    in0=qkvh[:, 0, :, h, 0:N][:, :, :, None].broadcast_to(
                        (P, FR, N, M16)),
                    in1=qkvh[:, 1, :, h][:, :, None, :].broadcast_to(
                        (P, FR, N, M16)),
                    op=OP.mult)
            ee = big.tile([P, FR, DIM, N, M16], BF16, tag="ee")
            nc.scalar.activation(out=ee, in_=e_pre,
                                 func=mybir.ActivationFunctionType.Exp)

            # ---- Z = sum_{n,m} E  (bf16 pairwise tree over m, then reduce) ----
            zt1 = work.tile([P, FR, DIM, N, 8], BF16, tag="zt1")
            nc.vector.tensor_tensor(out=zt1, in0=ee[:, :, :, :, 0:8],
                                    in1=ee[:, :, :, :, 8:16], op=OP.add)
            zt2 = work.tile([P, FR, DIM, N, 4], BF16, tag="zt2")
            nc.vector.tensor_tensor(out=zt2, in0=zt1[:, :, :, :, 0:4],
                                    in1=zt1[:, :, :, :, 4:8], op=OP.add)
            zt3 = work.tile([P, FR, DIM, N, 2], BF16, tag="zt3")
            nc.vector.tensor_tensor(out=zt3, in0=zt2[:, :, :, :, 0:2],
                                    in1=zt2[:, :, :, :, 2:4], op=OP.add)
            zz = work.tile([P, FR, DIM], F32, tag="zz")
            nc.vector.reduce_sum(out=zz, in_=zt3, axis=AX.XY)
            rz = work.tile([P, FR, DIM], F32, tag="rz")
            nc.vector.reciprocal(out=rz, in_=zz)

            # ---- weighted v: vz16 = vh*rz (bf16), vh16 = vh (bf16) ----
            vz16 = work.tile([P, FR, DIM, M16], BF16, tag="vz16")
            nc.vector.tensor_tensor(
                out=vz16, in0=qkvh[:, 2],
                in1=rz[:, :, :, None].broadcast_to((P, FR, DIM, M16)), op=OP.mult)
            vh16 = work.tile([P, FR, DIM, M16], BF16, tag="vh16")
            nc.scalar.copy(vh16, qkvh[:, 2])
            dist16 = work.tile([P, FR, N, M16], BF16, tag="dist16")
            nc.scalar.mul(dist16[:, :, :, 0:N], dt, 2.0)
            nc.vector.memset(dist16[:, :, :, N:M16], 0.0)

            # ---- P = E*vz16 + dist16*vh16, then pairwise m-tree ----
            p1 = big.tile([P, FR, DIM, N, M16], BF16, tag="p1")
            p2 = big.tile([P, FR, DIM, N, M16], BF16, tag="p2")
            for h in range(DIM):
                nc.vector.tensor_tensor(
                    out=p1[:, :, h], in0=ee[:, :, h],
                    in1=vz16[:, :, h][:, :, None, :].broadcast_to(
                        (P, FR, N, M16)),
                    op=OP.mult)
                nc.vector.tensor_tensor(
                    out=p2[:, :, h],
                    in0=dist16,
                    in1=vh16[:, :, h][:, :, None, :].broadcast_to(
                        (P, FR, N, M16)),
                    op=OP.mult)
            # fold the P1+P2 add into the first tree stage (3 adds of half
            # width instead of full-width add + first stage)
            pa = work.tile([P, FR, DIM, N, 8], BF16, tag="pa")
            nc.vector.tensor_tensor(out=pa, in0=p1[:, :, :, :, 0:8],
                                    in1=p2[:, :, :, :, 0:8], op=OP.add)
            pb = work.tile([P, FR, DIM, N, 8], BF16, tag="pb")
            nc.vector.tensor_tensor(out=pb, in0=p1[:, :, :, :, 8:16],
                                    in1=p2[:, :, :, :, 8:16], op=OP.add)
            pt1 = work.tile([P, FR, DIM, N, 8], BF16, tag="pt1")
            nc.vector.tensor_tensor(out=pt1, in0=pa, in1=pb, op=OP.add)
            pt2 = work.tile([P, FR, DIM, N, 4], BF16, tag="pt2")
            nc.vector.tensor_tensor(out=pt2, in0=pt1[:, :, :, :, 0:4],
                                    in1=pt1[:, :, :, :, 4:8], op=OP.add)
            pt3 = work.tile([P, FR, DIM, N, 2], BF16, tag="pt3")
            nc.vector.tensor_tensor(out=pt3, in0=pt2[:, :, :, :, 0:2],
                                    in1=pt2[:, :, :, :, 2:4], op=OP.add)
            att = work.tile([P, FR, DIM, N], F32, tag="att")
            nc.vector.reduce_sum(out=att, in_=pt3, axis=AX.X)

            # ---- output projection on PE: per 2-frame chunk ----
            att_f = att.rearrange("p f h n -> p (f h n)")       # [P, 180]
            outt = outp.tile([P, FR * FEAT], F32, tag="outt")   # (f, e)
            for c in range(FR // 2):
                at_ps = psum.tile([2 * FEAT, P], F32, tag="at_ps")
                nc.tensor.transpose(out=at_ps, in_=att_f[:, c * 90:(c + 1) * 90],
                                    identity=idn)
                at_sb = outp.tile([2 * FEAT, P], F32, tag="at_sb")
                nc.vector.tensor_copy(at_sb, at_ps)
                ot_ps = psum.tile([2 * FEAT, P], F32, tag="ot_ps")
                nc.tensor.matmul(out=ot_ps, lhsT=mo, rhs=at_sb,
                                 start=True, stop=True)
                ot_sb = outp.tile([2 * FEAT, P], F32, tag="ot_sb")
                nc.vector.tensor_scalar(out=ot_sb, in0=ot_ps, scalar1=bo2,
                                        scalar2=None, op0=OP.add)
                ob_ps = psum.tile([P, 2 * FEAT], F32, tag="ob_ps")
                nc.tensor.transpose(out=ob_ps, in_=ot_sb,
                                    identity=idn[0:2 * FEAT, 0:2 * FEAT])
                nc.vector.tensor_copy(outt[:, c * 90:(c + 1) * 90], ob_ps)

            nc.sync.dma_start(out=ov[it], in_=outt)


def _prep_consts(Wq, bq, Wk, bk, Wv, bv, Wo, bo):
    wc = np.stack([Wq, Wk, Wv]).astype(np.float32).reshape(-1)     # (t,h,d)
    bcv = np.stack([bq, bk, bv]).astype(np.float32).reshape(-1)    # (t,h)
    mo1 = np.zeros((FEAT, FEAT), np.float32)                        # [(h,n), e]
    for h in range(DIM):
        for n in range(N):
            mo1[h * N + n, :] = Wo[:, n * DIM + h]
    mo2 = np.zeros((2 * FEAT, 2 * FEAT), np.float32)
    mo2[:FEAT, :FEAT] = mo1
    mo2[FEAT:, FEAT:] = mo1
    bo2 = np.concatenate([bo, bo]).astype(np.float32)
    return wc, bcv, mo2, bo2


_NC_CACHE = {}


def _get_nc():
    if "nc" not in _NC_CACHE:
        _NC_CACHE["nc"] = build_nc()
    return _NC_CACHE["nc"]


def run(inputs, trace=False):
    q = np.ascontiguousarray(np.asarray(inputs["q"], np.float32).reshape(-1, FEAT))
    k = np.ascontiguousarray(np.asarray(inputs["k"], np.float32).reshape(-1, FEAT))
    v = np.ascontiguousarray(np.asarray(inputs["v"], np.float32).reshape(-1, FEAT))
    d = np.ascontiguousarray(
        np.asarray(inputs["distances"], np.float32).reshape(-1, N * N))
    wc, bcv, mo2, bo2 = _prep_consts(
        np.asarray(inputs["Wq"]), np.asarray(inputs["bq"]),
        np.asarray(inputs["Wk"]), np.asarray(inputs["bk"]),
        np.asarray(inputs["Wv"]), np.asarray(inputs["bv"]),
        np.asarray(inputs["Wo"]), np.asarray(inputs["bo"]))
    nc = _get_nc()
    in_maps = []
    for i in range(N_CORES):
        sl = slice(i * F_CORE, (i + 1) * F_CORE)
        in_maps.append({
            "q": q[sl], "k": k[sl], "v": v[sl], "d": d[sl],
            "wc": wc, "bc": bcv, "mo": mo2, "bo": bo2,
        })
    res = run_bass_kernel_spmd(nc, in_maps, list(range(N_CORES)), trace=trace)
    out = np.concatenate([res.results[i]["out"] for i in range(N_CORES)], axis=0)
    out = out.reshape(BS, SQ, FEAT)
    att_val = np.zeros((SQ,), dtype=np.float32)
    return (out, att_val), res


def kernel(**inputs):
    (out, att_val), _ = run(inputs, trace=False)
    return out, att_val


# revision 17
# speedup vs baseline: 1.0469x; 1.0469x over previous
"""Trainium2 Bass kernel for nn_MultiHeadAttention_spatial_shared.

Math (per frame f of BS*SQ=131072, all independent):
  qh[h,n] = sum_d Wq[h,d]*q[f, d*15+n] + bq[h]   (same for k, v)
  aq = |qh|, ak = |kh|
  E[h,n,m] = exp(aq[h,n]*ak[h,m])                (flattened softmax numerator)
  Z[h]     = sum_{n,m} E[h,n,m]
  att[h,n] = sum_m E[h,n,m]*vh[h,m]/Z[h] + 2*sum_m dist[n,m]*vh[h,m]
  out[e]   = sum_{h,n} Wo[e, n*3+h]*att[h,n] + bo[e]

Sharding: pure data parallel over frames across 8 cores (16384 frames/core).

On-chip layout: frames on partitions, 4 frames per partition ("FR"), 512
frames per super-tile, 32 super-tiles per core.  The m axis is padded to 16
(pad of ak is -1e30 so exp()=0; pad of v/dist is 0) so the m-reduction can be
done as a bf16 pairwise tree at 2x DVE rate.  The 45x45 output projection and
its transposes run on the TensorEngine.
"""

import numpy as np

import concourse.bass as bass
import concourse.bacc as bacc
import concourse.tile as tile
from concourse import mybir
from concourse.bass_utils import run_bass_kernel_spmd
from concourse.masks import make_identity

F32 = mybir.dt.float32
BF16 = mybir.dt.bfloat16
U32 = mybir.dt.uint32
AX = mybir.AxisListType
OP = mybir.AluOpType

N_CORES = 8
BS, SQ, DIM, N = 32, 4096, 3, 15
FEAT = DIM * N            # 45
M16 = 16                  # m padded to 16
P = 128                   # partitions
FR = 4                    # frames per partition per super-tile
TILE_F = P * FR           # 512 frames per super-tile
F_CORE = BS * SQ // N_CORES  # 16384


def build_nc(n_tiles=F_CORE // TILE_F):
    f_core = n_tiles * TILE_F
    nc = bacc.Bacc("TRN2", target_bir_lowering=False, debug=False)

    qx = nc.dram_tensor("q", [f_core, FEAT], F32, kind="ExternalInput").ap()
    kx = nc.dram_tensor("k", [f_core, FEAT], F32, kind="ExternalInput").ap()
    vx = nc.dram_tensor("v", [f_core, FEAT], F32, kind="ExternalInput").ap()
    dx = nc.dram_tensor("d", [f_core, N * N], F32, kind="ExternalInput").ap()
    wcx = nc.dram_tensor("wc", [3 * DIM * DIM], F32, kind="ExternalInput").ap()
    bcx = nc.dram_tensor("bc", [3 * DIM], F32, kind="ExternalInput").ap()
    mox = nc.dram_tensor("mo", [2 * FEAT, 2 * FEAT], F32, kind="ExternalInput").ap()
    box = nc.dram_tensor("bo", [2 * FEAT], F32, kind="ExternalInput").ap()
    ox = nc.dram_tensor("out", [f_core, FEAT], F32, kind="ExternalOutput").ap()

    # DRAM views: super-tile t, partition p, frame-in-partition f
    qv = qx.rearrange("(t p f) e -> t p (f e)", p=P, f=FR)
    kv = kx.rearrange("(t p f) e -> t p (f e)", p=P, f=FR)
    vv = vx.rearrange("(t p f) e -> t p (f e)", p=P, f=FR)
    dv = dx.rearrange("(t p f) e -> t p (f e)", p=P, f=FR)
    ov = ox.rearrange("(t p f) e -> t p (f e)", p=P, f=FR)

    with tile.TileContext(nc) as tc:
        _kernel_body(tc, n_tiles, qv, kv, vv, dv, ov, wcx, bcx, mox, box)
    nc.compile()
    return nc


def _kernel_body(tc, n_tiles, qv, kv, vv, dv, ov, wcx, bcx, mox, box):
    nc = tc.nc
    from contextlib import ExitStack
    with ExitStack() as ctx:
        consts = ctx.enter_context(tc.tile_pool(name="consts", bufs=1))
        dma_in = ctx.enter_context(tc.tile_pool(name="dma_in", bufs=3))
        work = ctx.enter_context(tc.tile_pool(name="work", bufs=2))
        big = ctx.enter_context(tc.tile_pool(name="big", bufs=2))
        outp = ctx.enter_context(tc.tile_pool(name="outp", bufs=3))
        psum = ctx.enter_context(tc.tile_pool(name="psum", bufs=2, space="PSUM"))

        def pbcast(ap, p=P):
            return bass.AP(tensor=ap.tensor, offset=ap.offset,
                           ap=[[0, p]] + list(ap.ap))

        # ---- constants (once) ----
        # DMA into staging tiles, then copy via DVE so downstream compute ops
        # only ever carry a single sync-wait class (walrus TT encoding allows
        # one wait per instruction).
        wc0 = consts.tile([P, 3, DIM, DIM], F32)       # (t, h, d)
        nc.sync.dma_start(out=wc0.rearrange("p a b c -> p (a b c)"),
                          in_=pbcast(wcx))
        wc = consts.tile([P, 3, DIM, DIM], F32)
        nc.vector.tensor_copy(wc, wc0)
        bc0 = consts.tile([P, 3, DIM], F32)            # (t, h)
        nc.sync.dma_start(out=bc0.rearrange("p a b -> p (a b)"),
                          in_=pbcast(bcx))
        bc = consts.tile([P, 3, DIM], F32)
        nc.vector.tensor_copy(bc, bc0)
        mo0 = consts.tile([2 * FEAT, 2 * FEAT], F32)   # blockdiag lhsT [(j,h,n),(j,e)]
        nc.sync.dma_start(out=mo0, in_=mox)
        mo = consts.tile([2 * FEAT, 2 * FEAT], F32)
        nc.vector.tensor_copy(mo, mo0)
        bo20 = consts.tile([2 * FEAT, 1], F32)
        nc.sync.dma_start(out=bo20, in_=box.rearrange("(p one) -> p one", one=1))
        bo2 = consts.tile([2 * FEAT, 1], F32)
        nc.vector.tensor_copy(bo2, bo20)
        idn0 = consts.tile([P, P], F32)
        make_identity(nc, idn0)
        idn = consts.tile([P, P], F32)
        nc.vector.tensor_copy(idn, idn0)

        for it in range(n_tiles):
            # ---- DMA in ----
            qt = dma_in.tile([P, FR, DIM, N], F32, tag="qt")
            kt = dma_in.tile([P, FR, DIM, N], F32, tag="kt")
            vt = dma_in.tile([P, FR, DIM, N], F32, tag="vt")
            dt = dma_in.tile([P, FR, N, N], F32, tag="dt")
            nc.sync.dma_start(out=qt.rearrange("p f d n -> p (f d n)"), in_=qv[it])
            nc.sync.dma_start(out=kt.rearrange("p f d n -> p (f d n)"), in_=kv[it])
            nc.sync.dma_start(out=vt.rearrange("p f d n -> p (f d n)"), in_=vv[it])
            nc.sync.dma_start(out=dt.rearrange("p f n m -> p (f n m)"), in_=dv[it])

            # ---- projections: qkvh[(t, fr, h, m16)] ----
            qkvh = work.tile([P, 3, FR, DIM, M16], F32, tag="qkvh")
            for t, src in enumerate((qt, kt, vt)):
                tmp = work.tile([P, FR, DIM, DIM, N], F32, tag="ptmp")
                nc.vector.tensor_tensor(
                    out=tmp,
                    in0=src[:, :, None, :, :].broadcast_to((P, FR, DIM, DIM, N)),
                    in1=wc[:, t][:, None, :, :, None].broadcast_to((P, FR, DIM, DIM, N)),
                    op=OP.mult)
                nc.vector.reduce_sum(
                    out=qkvh[:, t, :, :, 0:N],
                    in_=tmp.rearrange("p f h d n -> p f h n d"),
                    axis=AX.X)
            # bias: one op over a pre-merged (t,fr) view (3 free dims)
            qkvh_m = qkvh.rearrange("p t f h m -> p (t f) h m")
            bc_m = bc[:, :, None, :].broadcast_to((P, 3, FR, DIM)).rearrange(
                "p t f h -> p (t f) h")
            nc.vector.tensor_tensor(
                out=qkvh_m[:, :, :, 0:N],
                in0=qkvh_m[:, :, :, 0:N],
                in1=bc_m[:, :, :, None].broadcast_to((P, 3 * FR, DIM, N)),
                op=OP.add)
            # abs of qh, kh on the Scalar engine (only the 15 real columns)
            nc.scalar.activation(
                out=qkvh[:, 0:2, :, :, 0:N], in_=qkvh[:, 0:2, :, :, 0:N],
                func=mybir.ActivationFunctionType.Abs)
            # pads: k pad -> -1e30 (exp -> 0); v pad -> 0; q pad never read
            nc.vector.memset(qkvh[:, 1, :, :, N:M16], -1e30)
            nc.vector.memset(qkvh[:, 2, :, :, N:M16], 0.0)

            # ---- E_pre = aq (x) ak16,  E = exp(E_pre) in bf16 ----
            e_pre = big.tile([P, FR, DIM, N, M16], F32, tag="e_pre")
            for h in range(DIM):
                nc.vector.tensor_tensor(
                    out=e_pre[:, :, h],
                    in0=qkvh[:, 0, :, h, 0:N][:, :, :, None].broadcast_to(
                        (P, FR, N, M16)),
                    in1=qkvh[:, 1, :, h][:, :, None, :].broadcast_to(
                        (P, FR, N, M16)),
                    op=OP.mult)
            ee = big.tile([P, FR, DIM, N, M16], BF16, tag="ee")
            nc.scalar.activation(out=ee, in_=e_pre,
                                 func=mybir.ActivationFunctionType.Exp)

            # ---- Z = sum_{n,m} E  (bf16 pairwise tree over m, then reduce) ----
            zt1 = work.tile([P, FR, DIM, N, 8], BF16, tag="zt1")
            nc.vector.tensor_tensor(out=zt1, in0=ee[:, :, :, :, 0:8],
                                    in1=ee[:, :, :, :, 8:16], op=OP.add)
            zt2 = work.tile([P, FR, DIM, N, 4], BF16, tag="zt2")
            nc.vector.tensor_tensor(out=zt2, in0=zt1[:, :, :, :, 0:4],
                                    in1=zt1[:, :, :, :, 4:8], op=OP.add)
            zt3 = work.tile([P, FR, DIM, N, 2], BF16, tag="zt3")
            nc.vector.tensor_tensor(out=zt3, in0=zt2[:, :, :, :, 0:2],
                                    in1=zt2[:, :, :, :, 2:4], op=OP.add)
            zz = work.tile([P, FR, DIM], F32, tag="zz")
            nc.vector.reduce_sum(out=zz, in_=zt3, axis=AX.XY)
            rz = work.tile([P, FR, DIM], F32, tag="rz")
            nc.vector.reciprocal(out=rz, in_=zz)

            # ---- weighted v: vz16 = vh*rz (bf16), vh16 = vh (bf16) ----
            vz16 = work.tile([P, FR, DIM, M16], BF16, tag="vz16")
            nc.vector.tensor_tensor(
                out=vz16, in0=qkvh[:, 2],
                in1=rz[:, :, :, None].broadcast_to((P, FR, DIM, M16)), op=OP.mult)
            vh16 = work.tile([P, FR, DIM, M16], BF16, tag="vh16")
            nc.scalar.copy(vh16, qkvh[:, 2])
            dist16 = work.tile([P, FR, N, M16], BF16, tag="dist16")
            nc.scalar.mul(dist16[:, :, :, 0:N], dt, 2.0)
            nc.vector.memset(dist16[:, :, :, N:M16], 0.0)

            # ---- P = E*vz16 + dist16*vh16, then pairwise m-tree ----
            p1 = big.tile([P, FR, DIM, N, M16], BF16, tag="p1")
            p2 = big.tile([P, FR, DIM, N, M16], BF16, tag="p2")
            for h in range(DIM):
                nc.vector.tensor_tensor(
                    out=p1[:, :, h], in0=ee[:, :, h],
                    in1=vz16[:, :, h][:, :, None, :].broadcast_to(
                        (P, FR, N, M16)),
                    op=OP.mult)
                nc.vector.tensor_tensor(
                    out=p2[:, :, h],
                    in0=dist16,
                    in1=vh16[:, :, h][:, :, None, :].broadcast_to(
                        (P, FR, N, M16)),
                    op=OP.mult)
            # fold the P1+P2 add into the first tree stage (3 adds of half
            # width instead of full-width add + first stage)
            pa = work.tile([P, FR, DIM, N, 8], BF16, tag="pa")
            nc.vector.tensor_tensor(out=pa, in0=p1[:, :, :, :, 0:8],
                                    in1=p2[:, :, :, :, 0:8], op=OP.add)
            pb = work.tile([P, FR, DIM, N, 8], BF16, tag="pb")
            nc.vector.tensor_tensor(out=pb, in0=p1[:, :, :, :, 8:16],
                                    in1=p2[:, :, :, :, 8:16], op=OP.add)
            pt1 = work.tile([P, FR, DIM, N, 8], BF16, tag="pt1")
            nc.vector.tensor_tensor(out=pt1, in0=pa, in1=pb, op=OP.add)
            pt2 = work.tile([P, FR, DIM, N, 4], BF16, tag="pt2")
            nc.vector.tensor_tensor(out=pt2, in0=pt1[:, :, :, :, 0:4],
                                    in1=pt1[:, :, :, :, 4:8], op=OP.add)
            pt3 = work.tile([P, FR, DIM, N, 2], BF16, tag="pt3")
            nc.vector.tensor_tensor(out=pt3, in0=pt2[:, :, :, :, 0:2],
                                    in1=pt2[:, :, :, :, 2:4], op=OP.add)
            att = work.tile([P, FR, DIM, N], F32, tag="att")
            nc.vector.reduce_sum(out=att, in_=pt3, axis=AX.X)

            # ---- output projection on PE: per 2-frame chunk ----
            att_f = att.rearrange("p f h n -> p (f h n)")       # [P, 180]
            outt = outp.tile([P, FR * FEAT], F32, tag="outt")   # (f, e)
            for c in range(FR // 2):
                at_ps = psum.tile([2 * FEAT, P], F32, tag="at_ps")
                nc.tensor.transpose(out=at_ps, in_=att_f[:, c * 90:(c + 1) * 90],
                                    identity=idn)
                at_sb = outp.tile([2 * FEAT, P], F32, tag="at_sb")
                nc.vector.tensor_copy(at_sb, at_ps)
                ot_ps = psum.tile([2 * FEAT, P], F32, tag="ot_ps")
                nc.tensor.matmul(out=ot_ps, lhsT=mo, rhs=at_sb,
                                 start=True, stop=True)
                ot_sb = outp.tile([2 * FEAT, P], F32, tag="ot_sb")
                nc.vector.tensor_scalar(out=ot_sb, in0=ot_ps, scalar1=bo2,
                                        scalar2=None, op0=OP.add)
                ob_ps = psum.tile([P, 2 * FEAT], F32, tag="ob_ps")
                nc.tensor.transpose(out=ob_ps, in_=ot_sb,
                                    identity=idn[0:2 * FEAT, 0:2 * FEAT])
                nc.vector.tensor_copy(outt[:, c * 90:(c + 1) * 90], ob_ps)

            nc.sync.dma_start(out=ov[it], in_=outt)


def _prep_consts(Wq, bq, Wk, bk, Wv, bv, Wo, bo):
    wc = np.stack([Wq, Wk, Wv]).astype(np.float32).reshape(-1)     # (t,h,d)
    bcv = np.stack([bq, bk, bv]).astype(np.float32).reshape(-1)    # (t,h)
    mo1 = np.zeros((FEAT, FEAT), np.float32)                        # [(h,n), e]
    for h in range(DIM):
        for n in range(N):
            mo1[h * N + n, :] = Wo[:, n * DIM + h]
    mo2 = np.zeros((2 * FEAT, 2 * FEAT), np.float32)
    mo2[:FEAT, :FEAT] = mo1
    mo2[FEAT:, FEAT:] = mo1
    bo2 = np.concatenate([bo, bo]).astype(np.float32)
    return wc, bcv, mo2, bo2


_NC_CACHE = {}


def _get_nc():
    if "nc" not in _NC_CACHE:
        _NC_CACHE["nc"] = build_nc()
    return _NC_CACHE["nc"]


def run(inputs, trace=False, tmpdir=None):
    q = np.ascontiguousarray(np.asarray(inputs["q"], np.float32).reshape(-1, FEAT))
    k = np.ascontiguousarray(np.asarray(inputs["k"], np.float32).reshape(-1, FEAT))
    v = np.ascontiguousarray(np.asarray(inputs["v"], np.float32).reshape(-1, FEAT))
    d = np.ascontiguousarray(
        np.asarray(inputs["distances"], np.float32).reshape(-1, N * N))
    wc, bcv, mo2, bo2 = _prep_consts(
        np.asarray(inputs["Wq"]), np.asarray(inputs["bq"]),
        np.asarray(inputs["Wk"]), np.asarray(inputs["bk"]),
        np.asarray(inputs["Wv"]), np.asarray(inputs["bv"]),
        np.asarray(inputs["Wo"]), np.asarray(inputs["bo"]))
    nc = _get_nc()
    in_maps = []
    for i in range(N_CORES):
        sl = slice(i * F_CORE, (i + 1) * F_CORE)
        in_maps.append({
            "q": q[sl], "k": k[sl], "v": v[sl], "d": d[sl],
            "wc": wc, "bc": bcv, "mo": mo2, "bo": bo2,
        })
    res = run_bass_kernel_spmd(nc, in_maps, list(range(N_CORES)), trace=trace,
                               tmpdir=tmpdir)
    out = np.concatenate([res.results[i]["out"] for i in range(N_CORES)], axis=0)
    out = out.reshape(BS, SQ, FEAT)
    att_val = np.zeros((SQ,), dtype=np.float32)
    return (out, att_val), res


def kernel(**inputs):
    (out, att_val), _ = run(inputs, trace=False)
    return out, att_val


# revision 23
# speedup vs baseline: 1.1707x; 1.1183x over previous
"""Trainium2 Bass kernel for nn_MultiHeadAttention_spatial_shared.

Math (per frame f of BS*SQ=131072, all independent):
  qh[h,n] = sum_d Wq[h,d]*q[f, d*15+n] + bq[h]   (same for k, v)
  aq = |qh|, ak = |kh|
  E[h,n,m] = exp(aq[h,n]*ak[h,m])                (flattened softmax numerator)
  Z[h]     = sum_{n,m} E[h,n,m]
  att[h,n] = sum_m E[h,n,m]*vh[h,m]/Z[h] + 2*sum_m dist[n,m]*vh[h,m]
  out[e]   = sum_{h,n} Wo[e, n*3+h]*att[h,n] + bo[e]

Sharding: pure data parallel over frames across 8 cores (16384 frames/core).

On-chip layout: frames on partitions, 4 frames per partition ("FR"), 512
frames per super-tile, 32 super-tiles per core.  The m axis is padded to 16
(pad of ak is -1e30 so exp()=0; pad of v/dist is 0) so the m-reduction can be
done as a bf16 pairwise tree at 2x DVE rate.  The 45x45 output projection and
its transposes run on the TensorEngine.
"""

import numpy as np

import concourse.bass as bass
import concourse.bacc as bacc
import concourse.tile as tile
from concourse import mybir
from concourse.bass_utils import run_bass_kernel_spmd
from concourse.masks import make_identity

F32 = mybir.dt.float32
BF16 = mybir.dt.bfloat16
U32 = mybir.dt.uint32
AX = mybir.AxisListType
OP = mybir.AluOpType

N_CORES = 8
BS, SQ, DIM, N = 32, 4096, 3, 15
FEAT = DIM * N            # 45
M16 = 16                  # m padded to 16
P = 128                   # partitions
FR = 4                    # frames per partition per super-tile
TILE_F = P * FR           # 512 frames per super-tile
F_CORE = BS * SQ // N_CORES  # 16384


def build_nc(n_tiles=F_CORE // TILE_F):
    f_core = n_tiles * TILE_F
    nc = bacc.Bacc("TRN2", target_bir_lowering=False, debug=False)

    qx = nc.dram_tensor("q", [f_core, FEAT], F32, kind="ExternalInput").ap()
    kx = nc.dram_tensor("k", [f_core, FEAT], F32, kind="ExternalInput").ap()
    vx = nc.dram_tensor("v", [f_core, FEAT], F32, kind="ExternalInput").ap()
    dx = nc.dram_tensor("d", [f_core, N * N], F32, kind="ExternalInput").ap()
    wcx = nc.dram_tensor("wc", [3 * DIM * DIM], F32, kind="ExternalInput").ap()
    bcx = nc.dram_tensor("bc", [3 * FR * DIM], F32, kind="ExternalInput").ap()
    mox = nc.dram_tensor("mo", [2 * FEAT, 2 * FEAT], F32, kind="ExternalInput").ap()
    box = nc.dram_tensor("bo", [2 * FEAT], F32, kind="ExternalInput").ap()
    ox = nc.dram_tensor("out", [f_core, FEAT], F32, kind="ExternalOutput").ap()

    # DRAM views: super-tile t, partition p, frame-in-partition f
    qv = qx.rearrange("(t p f) e -> t p (f e)", p=P, f=FR)
    kv = kx.rearrange("(t p f) e -> t p (f e)", p=P, f=FR)
    vv = vx.rearrange("(t p f) e -> t p (f e)", p=P, f=FR)
    dv = dx.rearrange("(t p f) e -> t p (f e)", p=P, f=FR)
    ov = ox.rearrange("(t p f) e -> t p (f e)", p=P, f=FR)

    with tile.TileContext(nc) as tc:
        _kernel_body(tc, n_tiles, qv, kv, vv, dv, ov, wcx, bcx, mox, box)
    nc.compile()
    return nc


def _kernel_body(tc, n_tiles, qv, kv, vv, dv, ov, wcx, bcx, mox, box):
    nc = tc.nc
    from contextlib import ExitStack
    with ExitStack() as ctx:
        consts = ctx.enter_context(tc.tile_pool(name="consts", bufs=1))
        dma_in = ctx.enter_context(tc.tile_pool(name="dma_in", bufs=3))
        work = ctx.enter_context(tc.tile_pool(name="work", bufs=2))
        big = ctx.enter_context(tc.tile_pool(name="big", bufs=2))
        outp = ctx.enter_context(tc.tile_pool(name="outp", bufs=3))
        psum = ctx.enter_context(tc.tile_pool(name="psum", bufs=2, space="PSUM"))

        def pbcast(ap, p=P):
            return bass.AP(tensor=ap.tensor, offset=ap.offset,
                           ap=[[0, p]] + list(ap.ap))

        # ---- constants (once) ----
        # DMA into staging tiles, then copy via DVE so downstream compute ops
        # only ever carry a single sync-wait class (walrus TT encoding allows
        # one wait per instruction).
        wc0 = consts.tile([P, 3, DIM, DIM], F32)       # (t, h, d)
        nc.sync.dma_start(out=wc0.rearrange("p a b c -> p (a b c)"),
                          in_=pbcast(wcx))
        wc = consts.tile([P, 3, DIM, DIM], F32)
        nc.vector.tensor_copy(wc, wc0)
        bc0 = consts.tile([P, 3 * FR, DIM], F32)       # ((t,f), h)
        nc.sync.dma_start(out=bc0.rearrange("p a b -> p (a b)"),
                          in_=pbcast(bcx))
        bc = consts.tile([P, 3 * FR, DIM], F32)
        nc.vector.tensor_copy(bc, bc0)
        mo0 = consts.tile([2 * FEAT, 2 * FEAT], F32)   # blockdiag lhsT [(j,h,n),(j,e)]
        nc.sync.dma_start(out=mo0, in_=mox)
        mo = consts.tile([2 * FEAT, 2 * FEAT], F32)
        nc.vector.tensor_copy(mo, mo0)
        bo20 = consts.tile([2 * FEAT, 1], F32)
        nc.sync.dma_start(out=bo20, in_=box.rearrange("(p one) -> p one", one=1))
        bo2 = consts.tile([2 * FEAT, 1], F32)
        nc.vector.tensor_copy(bo2, bo20)
        idn0 = consts.tile([P, P], F32)
        make_identity(nc, idn0)
        idn = consts.tile([P, P], F32)
        nc.vector.tensor_copy(idn, idn0)

        for it in range(n_tiles):
            # ---- DMA in ----
            qt = dma_in.tile([P, FR, DIM, N], F32, tag="qt")
            kt = dma_in.tile([P, FR, DIM, N], F32, tag="kt")
            vt = dma_in.tile([P, FR, DIM, N], F32, tag="vt")
            dt = dma_in.tile([P, FR, N, N], F32, tag="dt")
            nc.sync.dma_start(out=qt.rearrange("p f d n -> p (f d n)"), in_=qv[it])
            nc.sync.dma_start(out=kt.rearrange("p f d n -> p (f d n)"), in_=kv[it])
            nc.sync.dma_start(out=vt.rearrange("p f d n -> p (f d n)"), in_=vv[it])
            nc.sync.dma_start(out=dt.rearrange("p f n m -> p (f n m)"), in_=dv[it])

            # ---- projections: qkvh[(t, fr, h, m16)] ----
            qkvh = work.tile([P, 3, FR, DIM, M16], F32, tag="qkvh")
            for t, src in enumerate((qt, kt, vt)):
                tmp = work.tile([P, FR, DIM, DIM, N], F32, tag="ptmp")
                nc.vector.tensor_tensor(
                    out=tmp,
                    in0=src[:, :, None, :, :].broadcast_to((P, FR, DIM, DIM, N)),
                    in1=wc[:, t][:, None, :, :, None].broadcast_to((P, FR, DIM, DIM, N)),
                    op=OP.mult)
                # d-contraction as two pairwise adds (cheaper than reduce)
                t01 = work.tile([P, FR, DIM, N], F32, tag="t01")
                nc.vector.tensor_tensor(
                    out=t01, in0=tmp[:, :, :, 0], in1=tmp[:, :, :, 1], op=OP.add)
                nc.vector.tensor_tensor(
                    out=qkvh[:, t, :, :, 0:N], in0=t01, in1=tmp[:, :, :, 2],
                    op=OP.add)
            # bias: one op over a pre-merged (t,fr) view (3 free dims)
            qkvh_m = qkvh.rearrange("p t f h m -> p (t f) h m")
            nc.vector.tensor_tensor(
                out=qkvh_m[:, :, :, 0:N],
                in0=qkvh_m[:, :, :, 0:N],
                in1=bc[:, :, :, None].broadcast_to((P, 3 * FR, DIM, N)),
                op=OP.add)
            # abs of qh, kh on the Scalar engine (only the 15 real columns)
            nc.scalar.activation(
                out=qkvh[:, 0:2, :, :, 0:N], in_=qkvh[:, 0:2, :, :, 0:N],
                func=mybir.ActivationFunctionType.Abs)
            # pads: k pad -> -1e30 (exp -> 0); v pad -> 0; q pad never read
            nc.vector.memset(qkvh[:, 1, :, :, N:M16], -1e30)
            nc.vector.memset(qkvh[:, 2, :, :, N:M16], 0.0)

            # ---- E_pre = aq (x) ak16,  E = exp(E_pre) in bf16 ----
            e_pre = big.tile([P, FR, DIM, N, M16], F32, tag="e_pre")
            for h in range(DIM):
                nc.vector.tensor_tensor(
                    out=e_pre[:, :, h],
                    in0=qkvh[:, 0, :, h, 0:N][:, :, :, None].broadcast_to(
                        (P, FR, N, M16)),
                    in1=qkvh[:, 1, :, h][:, :, None, :].broadcast_to(
                        (P, FR, N, M16)),
                    op=OP.mult)
            ee = big.tile([P, FR, DIM, N, M16], BF16, tag="ee")
            nc.scalar.activation(out=ee, in_=e_pre,
                                 func=mybir.ActivationFunctionType.Exp)

            # ---- Z = sum_{n,m} E  (bf16 pairwise tree over m, then reduce) ----
            zt1 = work.tile([P, FR, DIM, N, 8], BF16, tag="zt1")
            nc.vector.tensor_tensor(out=zt1, in0=ee[:, :, :, :, 0:8],
                                    in1=ee[:, :, :, :, 8:16], op=OP.add)
            zt2 = work.tile([P, FR, DIM, N, 4], BF16, tag="zt2")
            nc.vector.tensor_tensor(out=zt2, in0=zt1[:, :, :, :, 0:4],
                                    in1=zt1[:, :, :, :, 4:8], op=OP.add)
            zt3 = work.tile([P, FR, DIM, N, 2], BF16, tag="zt3")
            nc.vector.tensor_tensor(out=zt3, in0=zt2[:, :, :, :, 0:2],
                                    in1=zt2[:, :, :, :, 2:4], op=OP.add)
            zz = work.tile([P, FR, DIM], F32, tag="zz")
            nc.vector.reduce_sum(out=zz, in_=zt3, axis=AX.XY)
            rz = work.tile([P, FR, DIM], F32, tag="rz")
            nc.vector.reciprocal(out=rz, in_=zz)

            # ---- weighted v: vz16 = vh*rz (bf16), vh16 = vh (bf16) ----
            vz16 = work.tile([P, FR, DIM, M16], BF16, tag="vz16")
            nc.vector.tensor_tensor(
                out=vz16, in0=qkvh[:, 2],
                in1=rz[:, :, :, None].broadcast_to((P, FR, DIM, M16)), op=OP.mult)
            vh16 = work.tile([P, FR, DIM, M16], BF16, tag="vh16")
            nc.scalar.copy(vh16, qkvh[:, 2])
            dist16 = work.tile([P, FR, N, M16], BF16, tag="dist16")
            nc.scalar.mul(dist16[:, :, :, 0:N], dt, 2.0)
            nc.vector.memset(dist16[:, :, :, N:M16], 0.0)

            # ---- P = E*vz16 + dist16*vh16, then pairwise m-tree ----
            p1 = big.tile([P, FR, DIM, N, M16], BF16, tag="p1")
            p2 = big.tile([P, FR, DIM, N, M16], BF16, tag="p2")
            for h in range(DIM):
                nc.vector.tensor_tensor(
                    out=p1[:, :, h], in0=ee[:, :, h],
                    in1=vz16[:, :, h][:, :, None, :].broadcast_to(
                        (P, FR, N, M16)),
                    op=OP.mult)
                nc.vector.tensor_tensor(
                    out=p2[:, :, h],
                    in0=dist16,
                    in1=vh16[:, :, h][:, :, None, :].broadcast_to(
                        (P, FR, N, M16)),
                    op=OP.mult)
            # fold the P1+P2 add into the first tree stage (3 adds of half
            # width instead of full-width add + first stage)
            pa = work.tile([P, FR, DIM, N, 8], BF16, tag="pa")
            nc.vector.tensor_tensor(out=pa, in0=p1[:, :, :, :, 0:8],
                                    in1=p2[:, :, :, :, 0:8], op=OP.add)
            pb = work.tile([P, FR, DIM, N, 8], BF16, tag="pb")
            nc.vector.tensor_tensor(out=pb, in0=p1[:, :, :, :, 8:16],
                                    in1=p2[:, :, :, :, 8:16], op=OP.add)
            pt1 = work.tile([P, FR, DIM, N, 8], BF16, tag="pt1")
            nc.vector.tensor_tensor(out=pt1, in0=pa, in1=pb, op=OP.add)
            pt2 = work.tile([P, FR, DIM, N, 4], BF16, tag="pt2")
            nc.vector.tensor_tensor(out=pt2, in0=pt1[:, :, :, :, 0:4],
                                    in1=pt1[:, :, :, :, 4:8], op=OP.add)
            pt3 = work.tile([P, FR, DIM, N, 2], BF16, tag="pt3")
            nc.vector.tensor_tensor(out=pt3, in0=pt2[:, :, :, :, 0:2],
                                    in1=pt2[:, :, :, :, 2:4], op=OP.add)
            att = work.tile([P, FR, DIM, N], F32, tag="att")
            nc.vector.reduce_sum(out=att, in_=pt3, axis=AX.X)

            # ---- output projection on PE: per 2-frame chunk ----
            att_f = att.rearrange("p f h n -> p (f h n)")       # [P, 180]
            outt = outp.tile([P, FR * FEAT], F32, tag="outt")   # (f, e)
            for c in range(FR // 2):
                at_ps = psum.tile([2 * FEAT, P], F32, tag="at_ps")
                nc.tensor.transpose(out=at_ps, in_=att_f[:, c * 90:(c + 1) * 90],
                                    identity=idn)
                at_sb = outp.tile([2 * FEAT, P], F32, tag="at_sb")
                nc.scalar.copy(at_sb, at_ps)
                ot_ps = psum.tile([2 * FEAT, P], F32, tag="ot_ps")
                nc.tensor.matmul(out=ot_ps, lhsT=mo, rhs=at_sb,
                                 start=True, stop=True)
                ot_sb = outp.tile([2 * FEAT, P], F32, tag="ot_sb")
                nc.scalar.add(ot_sb, ot_ps, bo2)
                ob_ps = psum.tile([P, 2 * FEAT], F32, tag="ob_ps")
                nc.tensor.transpose(out=ob_ps, in_=ot_sb,
                                    identity=idn[0:2 * FEAT, 0:2 * FEAT])
                nc.scalar.copy(outt[:, c * 90:(c + 1) * 90], ob_ps)

            nc.sync.dma_start(out=ov[it], in_=outt)


def _prep_consts(Wq, bq, Wk, bk, Wv, bv, Wo, bo):
    wc = np.stack([Wq, Wk, Wv]).astype(np.float32).reshape(-1)     # (t,h,d)
    bcv = np.stack([bq, bk, bv]).astype(np.float32)                # [t, h]
    bcv = np.repeat(bcv[:, None, :], FR, axis=1).reshape(-1)       # (t,f,h)
    mo1 = np.zeros((FEAT, FEAT), np.float32)                        # [(h,n), e]
    for h in range(DIM):
        for n in range(N):
            mo1[h * N + n, :] = Wo[:, n * DIM + h]
    mo2 = np.zeros((2 * FEAT, 2 * FEAT), np.float32)
    mo2[:FEAT, :FEAT] = mo1
    mo2[FEAT:, FEAT:] = mo1
    bo2 = np.concatenate([bo, bo]).astype(np.float32)
    return wc, bcv, mo2, bo2


_NC_CACHE = {}


def _get_nc():
    if "nc" not in _NC_CACHE:
        _NC_CACHE["nc"] = build_nc()
    return _NC_CACHE["nc"]


def run(inputs, trace=False, tmpdir=None):
    q = np.ascontiguousarray(np.asarray(inputs["q"], np.float32).reshape(-1, FEAT))
    k = np.ascontiguousarray(np.asarray(inputs["k"], np.float32).reshape(-1, FEAT))
    v = np.ascontiguousarray(np.asarray(inputs["v"], np.float32).reshape(-1, FEAT))
    d = np.ascontiguousarray(
        np.asarray(inputs["distances"], np.float32).reshape(-1, N * N))
    wc, bcv, mo2, bo2 = _prep_consts(
        np.asarray(inputs["Wq"]), np.asarray(inputs["bq"]),
        np.asarray(inputs["Wk"]), np.asarray(inputs["bk"]),
        np.asarray(inputs["Wv"]), np.asarray(inputs["bv"]),
        np.asarray(inputs["Wo"]), np.asarray(inputs["bo"]))
    nc = _get_nc()
    in_maps = []
    for i in range(N_CORES):
        sl = slice(i * F_CORE, (i + 1) * F_CORE)
        in_maps.append({
            "q": q[sl], "k": k[sl], "v": v[sl], "d": d[sl],
            "wc": wc, "bc": bcv, "mo": mo2, "bo": bo2,
        })
    res = run_bass_kernel_spmd(nc, in_maps, list(range(N_CORES)), trace=trace,
                               tmpdir=tmpdir)
    out = np.concatenate([res.results[i]["out"] for i in range(N_CORES)], axis=0)
    out = out.reshape(BS, SQ, FEAT)
    att_val = np.zeros((SQ,), dtype=np.float32)
    return (out, att_val), res


def kernel(**inputs):
    (out, att_val), _ = run(inputs, trace=False)
    return out, att_val


# revision 24
# speedup vs baseline: 1.2834x; 1.0964x over previous
"""Trainium2 Bass kernel for nn_MultiHeadAttention_spatial_shared.

Math (per frame f of BS*SQ=131072, all independent):
  qh[h,n] = sum_d Wq[h,d]*q[f, d*15+n] + bq[h]   (same for k, v)
  aq = |qh|, ak = |kh|
  E[h,n,m] = exp(aq[h,n]*ak[h,m])                (flattened softmax numerator)
  Z[h]     = sum_{n,m} E[h,n,m]
  att[h,n] = sum_m E[h,n,m]*vh[h,m]/Z[h] + 2*sum_m dist[n,m]*vh[h,m]
  out[e]   = sum_{h,n} Wo[e, n*3+h]*att[h,n] + bo[e]

Sharding: pure data parallel over frames across 8 cores (16384 frames/core).

On-chip layout: frames on partitions, 4 frames per partition ("FR"), 512
frames per super-tile, 32 super-tiles per core.  The m axis is padded to 16
(pad of ak is -1e30 so exp()=0; pad of v/dist is 0) so the m-reduction can be
done as a bf16 pairwise tree at 2x DVE rate.  The 45x45 output projection and
its transposes run on the TensorEngine.
"""

import numpy as np

import concourse.bass as bass
import concourse.bacc as bacc
import concourse.tile as tile
from concourse import mybir
from concourse.bass_utils import run_bass_kernel_spmd
from concourse.masks import make_identity

F32 = mybir.dt.float32
BF16 = mybir.dt.bfloat16
U32 = mybir.dt.uint32
AX = mybir.AxisListType
OP = mybir.AluOpType

N_CORES = 8
BS, SQ, DIM, N = 32, 4096, 3, 15
FEAT = DIM * N            # 45
M16 = 16                  # m padded to 16
P = 128                   # partitions
FR = 4                    # frames per partition per super-tile
TILE_F = P * FR           # 512 frames per super-tile
F_CORE = BS * SQ // N_CORES  # 16384


def build_nc(n_tiles=F_CORE // TILE_F):
    f_core = n_tiles * TILE_F
    nc = bacc.Bacc("TRN2", target_bir_lowering=False, debug=False)

    qx = nc.dram_tensor("q", [f_core, FEAT], F32, kind="ExternalInput").ap()
    kx = nc.dram_tensor("k", [f_core, FEAT], F32, kind="ExternalInput").ap()
    vx = nc.dram_tensor("v", [f_core, FEAT], F32, kind="ExternalInput").ap()
    dx = nc.dram_tensor("d", [f_core, N * N], F32, kind="ExternalInput").ap()
    wcx = nc.dram_tensor("wc", [3 * DIM * DIM], F32, kind="ExternalInput").ap()
    bcx = nc.dram_tensor("bc", [3 * FR * DIM], F32, kind="ExternalInput").ap()
    mox = nc.dram_tensor("mo", [2 * FEAT, 2 * FEAT], F32, kind="ExternalInput").ap()
    box = nc.dram_tensor("bo", [2 * FEAT], F32, kind="ExternalInput").ap()
    ox = nc.dram_tensor("out", [f_core, FEAT], F32, kind="ExternalOutput").ap()

    # DRAM views: super-tile t, partition p, frame-in-partition f
    qv = qx.rearrange("(t p f) e -> t p (f e)", p=P, f=FR)
    kv = kx.rearrange("(t p f) e -> t p (f e)", p=P, f=FR)
    vv = vx.rearrange("(t p f) e -> t p (f e)", p=P, f=FR)
    dv = dx.rearrange("(t p f) e -> t p (f e)", p=P, f=FR)
    ov = ox.rearrange("(t p f) e -> t p (f e)", p=P, f=FR)

    with tile.TileContext(nc) as tc:
        _kernel_body(tc, n_tiles, qv, kv, vv, dv, ov, wcx, bcx, mox, box)
    nc.compile()
    return nc


def _kernel_body(tc, n_tiles, qv, kv, vv, dv, ov, wcx, bcx, mox, box):
    nc = tc.nc
    from contextlib import ExitStack
    with ExitStack() as ctx:
        consts = ctx.enter_context(tc.tile_pool(name="consts", bufs=1))
        dma_in = ctx.enter_context(tc.tile_pool(name="dma_in", bufs=3))
        work = ctx.enter_context(tc.tile_pool(name="work", bufs=2))
        big = ctx.enter_context(tc.tile_pool(name="big", bufs=2))
        outp = ctx.enter_context(tc.tile_pool(name="outp", bufs=3))
        psum = ctx.enter_context(tc.tile_pool(name="psum", bufs=2, space="PSUM"))

        def pbcast(ap, p=P):
            return bass.AP(tensor=ap.tensor, offset=ap.offset,
                           ap=[[0, p]] + list(ap.ap))

        # ---- constants (once) ----
        # DMA into staging tiles, then copy via DVE so downstream compute ops
        # only ever carry a single sync-wait class (walrus TT encoding allows
        # one wait per instruction).
        wc0 = consts.tile([P, 3, DIM, DIM], F32)       # (t, h, d)
        nc.sync.dma_start(out=wc0.rearrange("p a b c -> p (a b c)"),
                          in_=pbcast(wcx))
        wc = consts.tile([P, 3, DIM, DIM], F32)
        nc.vector.tensor_copy(wc, wc0)
        bc0 = consts.tile([P, 3 * FR, DIM], F32)       # ((t,f), h)
        nc.sync.dma_start(out=bc0.rearrange("p a b -> p (a b)"),
                          in_=pbcast(bcx))
        bc = consts.tile([P, 3 * FR, DIM], F32)
        nc.vector.tensor_copy(bc, bc0)
        mo0 = consts.tile([2 * FEAT, 2 * FEAT], F32)   # blockdiag lhsT [(j,h,n),(j,e)]
        nc.sync.dma_start(out=mo0, in_=mox)
        mo = consts.tile([2 * FEAT, 2 * FEAT], F32)
        nc.vector.tensor_copy(mo, mo0)
        bo20 = consts.tile([2 * FEAT, 1], F32)
        nc.sync.dma_start(out=bo20, in_=box.rearrange("(p one) -> p one", one=1))
        bo2 = consts.tile([2 * FEAT, 1], F32)
        nc.vector.tensor_copy(bo2, bo20)
        idn0 = consts.tile([P, P], F32)
        make_identity(nc, idn0)
        idn = consts.tile([P, P], F32)
        nc.vector.tensor_copy(idn, idn0)

        for it in range(n_tiles):
            # ---- DMA in ----
            qt = dma_in.tile([P, FR, DIM, N], F32, tag="qt")
            kt = dma_in.tile([P, FR, DIM, N], F32, tag="kt")
            vt = dma_in.tile([P, FR, DIM, N], F32, tag="vt")
            dt = dma_in.tile([P, FR, N, N], F32, tag="dt")
            nc.sync.dma_start(out=qt.rearrange("p f d n -> p (f d n)"), in_=qv[it])
            nc.sync.dma_start(out=kt.rearrange("p f d n -> p (f d n)"), in_=kv[it])
            nc.sync.dma_start(out=vt.rearrange("p f d n -> p (f d n)"), in_=vv[it])
            nc.sync.dma_start(out=dt.rearrange("p f n m -> p (f n m)"), in_=dv[it])

            # ---- projections: qkvh[(t, fr, h, m16)] ----
            qkvh = work.tile([P, 3, FR, DIM, M16], F32, tag="qkvh")
            for t, src in enumerate((qt, kt, vt)):
                tmp = work.tile([P, FR, DIM, DIM, N], F32, tag="ptmp")
                nc.vector.tensor_tensor(
                    out=tmp,
                    in0=src[:, :, None, :, :].broadcast_to((P, FR, DIM, DIM, N)),
                    in1=wc[:, t][:, None, :, :, None].broadcast_to((P, FR, DIM, DIM, N)),
                    op=OP.mult)
                # d-contraction as two pairwise adds (cheaper than reduce)
                t01 = work.tile([P, FR, DIM, N], F32, tag="t01")
                nc.vector.tensor_tensor(
                    out=t01, in0=tmp[:, :, :, 0], in1=tmp[:, :, :, 1], op=OP.add)
                nc.vector.tensor_tensor(
                    out=qkvh[:, t, :, :, 0:N], in0=t01, in1=tmp[:, :, :, 2],
                    op=OP.add)
            # bias: one op over a pre-merged (t,fr) view (3 free dims)
            qkvh_m = qkvh.rearrange("p t f h m -> p (t f) h m")
            nc.vector.tensor_tensor(
                out=qkvh_m[:, :, :, 0:N],
                in0=qkvh_m[:, :, :, 0:N],
                in1=bc[:, :, :, None].broadcast_to((P, 3 * FR, DIM, N)),
                op=OP.add)
            # abs of qh on ScalarE (15 real columns, in place, f32)
            nc.scalar.activation(
                out=qkvh[:, 0, :, :, 0:N], in_=qkvh[:, 0, :, :, 0:N],
                func=mybir.ActivationFunctionType.Abs)
            # ak as bf16 with abs fused into the cast; pad -> -1e30 (exp -> 0)
            ak16 = work.tile([P, FR, DIM, M16], BF16, tag="ak16")
            nc.scalar.activation(
                out=ak16[:, :, :, 0:N], in_=qkvh[:, 1, :, :, 0:N],
                func=mybir.ActivationFunctionType.Abs)
            nc.vector.memset(ak16[:, :, :, N:M16], -1e30)
            # v pad -> 0 (read by vz16/vh16)
            nc.vector.memset(qkvh[:, 2, :, :, N:M16], 0.0)

            # ---- E_pre = aq (x) ak16 in bf16 (2x DVE via materialized
            # broadcast of aq), E = exp(E_pre) ----
            aqb = big.tile([P, FR, DIM, N, M16], BF16, tag="aqb")
            for h in range(DIM):
                nc.scalar.copy(
                    aqb[:, :, h],
                    qkvh[:, 0, :, h, 0:N][:, :, :, None].broadcast_to(
                        (P, FR, N, M16)))
            e_pre = big.tile([P, FR, DIM, N, M16], BF16, tag="e_pre")
            for h in range(DIM):
                nc.vector.tensor_tensor(
                    out=e_pre[:, :, h],
                    in0=aqb[:, :, h],
                    in1=ak16[:, :, h][:, :, None, :].broadcast_to(
                        (P, FR, N, M16)),
                    op=OP.mult)
            ee = big.tile([P, FR, DIM, N, M16], BF16, tag="ee")
            nc.scalar.activation(out=ee, in_=e_pre,
                                 func=mybir.ActivationFunctionType.Exp)

            # ---- Z = sum_{n,m} E  (bf16 pairwise tree over m, then reduce) ----
            zt1 = work.tile([P, FR, DIM, N, 8], BF16, tag="zt1")
            nc.vector.tensor_tensor(out=zt1, in0=ee[:, :, :, :, 0:8],
                                    in1=ee[:, :, :, :, 8:16], op=OP.add)
            zt2 = work.tile([P, FR, DIM, N, 4], BF16, tag="zt2")
            nc.vector.tensor_tensor(out=zt2, in0=zt1[:, :, :, :, 0:4],
                                    in1=zt1[:, :, :, :, 4:8], op=OP.add)
            zt3 = work.tile([P, FR, DIM, N, 2], BF16, tag="zt3")
            nc.vector.tensor_tensor(out=zt3, in0=zt2[:, :, :, :, 0:2],
                                    in1=zt2[:, :, :, :, 2:4], op=OP.add)
            zz = work.tile([P, FR, DIM], F32, tag="zz")
            nc.vector.reduce_sum(out=zz, in_=zt3, axis=AX.XY)
            rz = work.tile([P, FR, DIM], F32, tag="rz")
            nc.vector.reciprocal(out=rz, in_=zz)

            # ---- weighted v: vz16 = vh*rz (bf16), vh16 = vh (bf16) ----
            vz16 = work.tile([P, FR, DIM, M16], BF16, tag="vz16")
            nc.vector.tensor_tensor(
                out=vz16, in0=qkvh[:, 2],
                in1=rz[:, :, :, None].broadcast_to((P, FR, DIM, M16)), op=OP.mult)
            vh16 = work.tile([P, FR, DIM, M16], BF16, tag="vh16")
            nc.scalar.copy(vh16, qkvh[:, 2])
            dist16 = work.tile([P, FR, N, M16], BF16, tag="dist16")
            nc.scalar.mul(dist16[:, :, :, 0:N], dt, 2.0)
            nc.vector.memset(dist16[:, :, :, N:M16], 0.0)

            # ---- P = E*vz16 + dist16*vh16, then pairwise m-tree ----
            p1 = big.tile([P, FR, DIM, N, M16], BF16, tag="p1")
            p2 = big.tile([P, FR, DIM, N, M16], BF16, tag="p2")
            for h in range(DIM):
                nc.vector.tensor_tensor(
                    out=p1[:, :, h], in0=ee[:, :, h],
                    in1=vz16[:, :, h][:, :, None, :].broadcast_to(
                        (P, FR, N, M16)),
                    op=OP.mult)
                nc.vector.tensor_tensor(
                    out=p2[:, :, h],
                    in0=dist16,
                    in1=vh16[:, :, h][:, :, None, :].broadcast_to(
                        (P, FR, N, M16)),
                    op=OP.mult)
            # fold the P1+P2 add into the first tree stage (3 adds of half
            # width instead of full-width add + first stage)
            pa = work.tile([P, FR, DIM, N, 8], BF16, tag="pa")
            nc.vector.tensor_tensor(out=pa, in0=p1[:, :, :, :, 0:8],
                                    in1=p2[:, :, :, :, 0:8], op=OP.add)
            pb = work.tile([P, FR, DIM, N, 8], BF16, tag="pb")
            nc.vector.tensor_tensor(out=pb, in0=p1[:, :, :, :, 8:16],
                                    in1=p2[:, :, :, :, 8:16], op=OP.add)
            pt1 = work.tile([P, FR, DIM, N, 8], BF16, tag="pt1")
            nc.vector.tensor_tensor(out=pt1, in0=pa, in1=pb, op=OP.add)
            pt2 = work.tile([P, FR, DIM, N, 4], BF16, tag="pt2")
            nc.vector.tensor_tensor(out=pt2, in0=pt1[:, :, :, :, 0:4],
                                    in1=pt1[:, :, :, :, 4:8], op=OP.add)
            pt3 = work.tile([P, FR, DIM, N, 2], BF16, tag="pt3")
            nc.vector.tensor_tensor(out=pt3, in0=pt2[:, :, :, :, 0:2],
                                    in1=pt2[:, :, :, :, 2:4], op=OP.add)
            att = work.tile([P, FR, DIM, N], F32, tag="att")
            nc.vector.reduce_sum(out=att, in_=pt3, axis=AX.X)

            # ---- output projection on PE: per 2-frame chunk ----
            att_f = att.rearrange("p f h n -> p (f h n)")       # [P, 180]
            outt = outp.tile([P, FR * FEAT], F32, tag="outt")   # (f, e)
            for c in range(FR // 2):
                at_ps = psum.tile([2 * FEAT, P], F32, tag="at_ps")
                nc.tensor.transpose(out=at_ps, in_=att_f[:, c * 90:(c + 1) * 90],
                                    identity=idn)
                at_sb = outp.tile([2 * FEAT, P], F32, tag="at_sb")
                nc.scalar.copy(at_sb, at_ps)
                ot_ps = psum.tile([2 * FEAT, P], F32, tag="ot_ps")
                nc.tensor.matmul(out=ot_ps, lhsT=mo, rhs=at_sb,
                                 start=True, stop=True)
                ot_sb = outp.tile([2 * FEAT, P], F32, tag="ot_sb")
                nc.scalar.add(ot_sb, ot_ps, bo2)
                ob_ps = psum.tile([P, 2 * FEAT], F32, tag="ob_ps")
                nc.tensor.transpose(out=ob_ps, in_=ot_sb,
                                    identity=idn[0:2 * FEAT, 0:2 * FEAT])
                nc.scalar.copy(outt[:, c * 90:(c + 1) * 90], ob_ps)

            nc.sync.dma_start(out=ov[it], in_=outt)


def _prep_consts(Wq, bq, Wk, bk, Wv, bv, Wo, bo):
    wc = np.stack([Wq, Wk, Wv]).astype(np.float32).reshape(-1)     # (t,h,d)
    bcv = np.stack([bq, bk, bv]).astype(np.float32)                # [t, h]
    bcv = np.repeat(bcv[:, None, :], FR, axis=1).reshape(-1)       # (t,f,h)
    mo1 = np.zeros((FEAT, FEAT), np.float32)                        # [(h,n), e]
    for h in range(DIM):
        for n in range(N):
            mo1[h * N + n, :] = Wo[:, n * DIM + h]
    mo2 = np.zeros((2 * FEAT, 2 * FEAT), np.float32)
    mo2[:FEAT, :FEAT] = mo1
    mo2[FEAT:, FEAT:] = mo1
    bo2 = np.concatenate([bo, bo]).astype(np.float32)
    return wc, bcv, mo2, bo2


_NC_CACHE = {}


def _get_nc():
    if "nc" not in _NC_CACHE:
        _NC_CACHE["nc"] = build_nc()
    return _NC_CACHE["nc"]


def run(inputs, trace=False, tmpdir=None):
    q = np.ascontiguousarray(np.asarray(inputs["q"], np.float32).reshape(-1, FEAT))
    k = np.ascontiguousarray(np.asarray(inputs["k"], np.float32).reshape(-1, FEAT))
    v = np.ascontiguousarray(np.asarray(inputs["v"], np.float32).reshape(-1, FEAT))
    d = np.ascontiguousarray(
        np.asarray(inputs["distances"], np.float32).reshape(-1, N * N))
    wc, bcv, mo2, bo2 = _prep_consts(
        np.asarray(inputs["Wq"]), np.asarray(inputs["bq"]),
        np.asarray(inputs["Wk"]), np.asarray(inputs["bk"]),
        np.asarray(inputs["Wv"]), np.asarray(inputs["bv"]),
        np.asarray(inputs["Wo"]), np.asarray(inputs["bo"]))
    nc = _get_nc()
    in_maps = []
    for i in range(N_CORES):
        sl = slice(i * F_CORE, (i + 1) * F_CORE)
        in_maps.append({
            "q": q[sl], "k": k[sl], "v": v[sl], "d": d[sl],
            "wc": wc, "bc": bcv, "mo": mo2, "bo": bo2,
        })
    res = run_bass_kernel_spmd(nc, in_maps, list(range(N_CORES)), trace=trace,
                               tmpdir=tmpdir)
    out = np.concatenate([res.results[i]["out"] for i in range(N_CORES)], axis=0)
    out = out.reshape(BS, SQ, FEAT)
    att_val = np.zeros((SQ,), dtype=np.float32)
    return (out, att_val), res


def kernel(**inputs):
    (out, att_val), _ = run(inputs, trace=False)
    return out, att_val


# revision 31
# speedup vs baseline: 1.3237x; 1.0313x over previous
"""Trainium2 Bass kernel for nn_MultiHeadAttention_spatial_shared.

Math (per frame f of BS*SQ=131072, all independent):
  qh[h,n] = sum_d Wq[h,d]*q[f, d*15+n] + bq[h]   (same for k, v)
  aq = |qh|, ak = |kh|
  E[h,n,m] = exp(aq[h,n]*ak[h,m])                (flattened softmax numerator)
  Z[h]     = sum_{n,m} E[h,n,m]
  att[h,n] = sum_m E[h,n,m]*vh[h,m]/Z[h] + 2*sum_m dist[n,m]*vh[h,m]
  out[e]   = sum_{h,n} Wo[e, n*3+h]*att[h,n] + bo[e]

Sharding: pure data parallel over frames across 8 cores (16384 frames/core).

On-chip layout: frames on partitions, 4 frames per partition ("FR"), 512
frames per super-tile, 32 super-tiles per core.  The m axis is padded to 16
(pad of ak is -1e30 so exp()=0; pad of v/dist is 0) so the m-reduction can be
done as a bf16 pairwise tree at 2x DVE rate.  The 45x45 output projection and
its transposes run on the TensorEngine.
"""

import numpy as np

import concourse.bass as bass
import concourse.bacc as bacc
import concourse.tile as tile
from concourse import mybir
from concourse.bass_utils import run_bass_kernel_spmd
from concourse.masks import make_identity

F32 = mybir.dt.float32
BF16 = mybir.dt.bfloat16
U32 = mybir.dt.uint32
AX = mybir.AxisListType
OP = mybir.AluOpType

N_CORES = 8
BS, SQ, DIM, N = 32, 4096, 3, 15
FEAT = DIM * N            # 45
M16 = 16                  # m padded to 16
P = 128                   # partitions
FR = 4                    # frames per partition per super-tile
TILE_F = P * FR           # 512 frames per super-tile
F_CORE = BS * SQ // N_CORES  # 16384


def build_nc(n_tiles=F_CORE // TILE_F):
    f_core = n_tiles * TILE_F
    nc = bacc.Bacc("TRN2", target_bir_lowering=False, debug=False)

    qx = nc.dram_tensor("q", [f_core, FEAT], F32, kind="ExternalInput").ap()
    kx = nc.dram_tensor("k", [f_core, FEAT], F32, kind="ExternalInput").ap()
    vx = nc.dram_tensor("v", [f_core, FEAT], F32, kind="ExternalInput").ap()
    dx = nc.dram_tensor("d", [f_core, N * N], F32, kind="ExternalInput").ap()
    wcx = nc.dram_tensor("wc", [3 * FR * DIM * DIM], F32,
                         kind="ExternalInput").ap()
    bcx = nc.dram_tensor("bc", [3 * FR * DIM], F32, kind="ExternalInput").ap()
    mox = nc.dram_tensor("mo", [2 * FEAT, 2 * FEAT], F32, kind="ExternalInput").ap()
    box = nc.dram_tensor("bo", [2 * FEAT], F32, kind="ExternalInput").ap()
    ox = nc.dram_tensor("out", [f_core, FEAT], F32, kind="ExternalOutput").ap()

    # DRAM views: super-tile t, partition p, frame-in-partition f
    qv = qx.rearrange("(t p f) e -> t p (f e)", p=P, f=FR)
    kv = kx.rearrange("(t p f) e -> t p (f e)", p=P, f=FR)
    vv = vx.rearrange("(t p f) e -> t p (f e)", p=P, f=FR)
    dv = dx.rearrange("(t p f) e -> t p (f e)", p=P, f=FR)
    ov = ox.rearrange("(t p f) e -> t p (f e)", p=P, f=FR)

    with tile.TileContext(nc) as tc:
        _kernel_body(tc, n_tiles, qv, kv, vv, dv, ov, wcx, bcx, mox, box)
    nc.compile()
    return nc


def _kernel_body(tc, n_tiles, qv, kv, vv, dv, ov, wcx, bcx, mox, box):
    nc = tc.nc
    from contextlib import ExitStack
    with ExitStack() as ctx:
        consts = ctx.enter_context(tc.tile_pool(name="consts", bufs=1))
        dma_in = ctx.enter_context(tc.tile_pool(name="dma_in", bufs=3))
        work = ctx.enter_context(tc.tile_pool(name="work", bufs=2))
        big = ctx.enter_context(tc.tile_pool(name="big", bufs=2))
        outp = ctx.enter_context(tc.tile_pool(name="outp", bufs=3))
        psum = ctx.enter_context(tc.tile_pool(name="psum", bufs=2, space="PSUM"))

        def pbcast(ap, p=P):
            return bass.AP(tensor=ap.tensor, offset=ap.offset,
                           ap=[[0, p]] + list(ap.ap))

        # ---- constants (once) ----
        # DMA into staging tiles, then copy via DVE so downstream compute ops
        # only ever carry a single sync-wait class (walrus TT encoding allows
        # one wait per instruction).
        wc0 = consts.tile([P, 3 * FR, DIM, DIM], F32)  # ((t,f), h, d)
        nc.sync.dma_start(out=wc0.rearrange("p a b c -> p (a b c)"),
                          in_=pbcast(wcx))
        wc = consts.tile([P, 3 * FR, DIM, DIM], F32)
        nc.vector.tensor_copy(wc, wc0)
        bc0 = consts.tile([P, 3 * FR, DIM], F32)       # ((t,f), h)
        nc.sync.dma_start(out=bc0.rearrange("p a b -> p (a b)"),
                          in_=pbcast(bcx))
        bc = consts.tile([P, 3 * FR, DIM], F32)
        nc.vector.tensor_copy(bc, bc0)
        mo0 = consts.tile([2 * FEAT, 2 * FEAT], F32)   # blockdiag lhsT [(j,h,n),(j,e)]
        nc.sync.dma_start(out=mo0, in_=mox)
        mo = consts.tile([2 * FEAT, 2 * FEAT], F32)
        nc.vector.tensor_copy(mo, mo0)
        bo20 = consts.tile([2 * FEAT, 1], F32)
        nc.sync.dma_start(out=bo20, in_=box.rearrange("(p one) -> p one", one=1))
        bo2 = consts.tile([2 * FEAT, 1], F32)
        nc.vector.tensor_copy(bo2, bo20)
        idn0 = consts.tile([P, P], F32)
        make_identity(nc, idn0)
        idn = consts.tile([P, P], F32)
        nc.vector.tensor_copy(idn, idn0)

        for it in range(n_tiles):
            # ---- DMA in: q,k,v land in one (t,f)-major tile ----
            qkv = dma_in.tile([P, 3 * FR, DIM, N], F32, tag="qkv")
            dt = dma_in.tile([P, FR, N, N], F32, tag="dt")
            nc.sync.dma_start(
                out=qkv[:, 0:FR].rearrange("p f d n -> p (f d n)"), in_=qv[it])
            nc.sync.dma_start(
                out=qkv[:, FR:2 * FR].rearrange("p f d n -> p (f d n)"),
                in_=kv[it])
            nc.sync.dma_start(
                out=qkv[:, 2 * FR:3 * FR].rearrange("p f d n -> p (f d n)"),
                in_=vv[it])
            nc.sync.dma_start(out=dt.rearrange("p f n m -> p (f n m)"), in_=dv[it])

            # ---- projections: qkvh[((t,f), h, m16)] ----
            qkvh = work.tile([P, 3 * FR, DIM, M16], F32, tag="qkvh")
            tmp = work.tile([P, 3 * FR, DIM, DIM, N], F32, tag="ptmp")
            for h in range(DIM):
                nc.vector.tensor_tensor(
                    out=tmp[:, :, h],
                    in0=qkv,
                    in1=wc[:, :, h][:, :, :, None].broadcast_to(
                        (P, 3 * FR, DIM, N)),
                    op=OP.mult)
            # d-contraction as two pairwise adds over all (t,f,h) at once
            t01 = work.tile([P, 3 * FR, DIM, N], F32, tag="t01")
            nc.vector.tensor_tensor(
                out=t01, in0=tmp[:, :, :, 0], in1=tmp[:, :, :, 1], op=OP.add)
            nc.vector.tensor_tensor(
                out=qkvh[:, :, :, 0:N], in0=t01, in1=tmp[:, :, :, 2], op=OP.add)
            # bias
            nc.vector.tensor_tensor(
                out=qkvh[:, :, :, 0:N],
                in0=qkvh[:, :, :, 0:N],
                in1=bc[:, :, :, None].broadcast_to((P, 3 * FR, DIM, N)),
                op=OP.add)
            # abs of qh on ScalarE (15 real columns, in place, f32)
            nc.scalar.activation(
                out=qkvh[:, 0:FR, :, 0:N], in_=qkvh[:, 0:FR, :, 0:N],
                func=mybir.ActivationFunctionType.Abs)
            # ak as bf16 with abs fused into the cast; pad -> -1e30 (exp -> 0)
            ak16 = work.tile([P, FR, DIM, M16], BF16, tag="ak16")
            nc.scalar.activation(
                out=ak16[:, :, :, 0:N], in_=qkvh[:, FR:2 * FR, :, 0:N],
                func=mybir.ActivationFunctionType.Abs)
            nc.vector.memset(ak16[:, :, :, N:M16], -1e30)
            # v pad -> 0 (read by vz16/vh16)
            nc.vector.memset(qkvh[:, 2 * FR:3 * FR, :, N:M16], 0.0)

            # ---- E_pre = aq (x) ak16 in bf16 (2x DVE via materialized
            # broadcast of aq), E = exp(E_pre) ----
            aqb = big.tile([P, FR, DIM, N, M16], BF16, tag="aqb")
            for h in range(DIM):
                nc.scalar.copy(
                    aqb[:, :, h],
                    qkvh[:, 0:FR, h, 0:N][:, :, :, None].broadcast_to(
                        (P, FR, N, M16)))
            e_pre = big.tile([P, FR, DIM, N, M16], BF16, tag="e_pre")
            for h in range(DIM):
                nc.vector.tensor_tensor(
                    out=e_pre[:, :, h],
                    in0=aqb[:, :, h],
                    in1=ak16[:, :, h][:, :, None, :].broadcast_to(
                        (P, FR, N, M16)),
                    op=OP.mult)
            ee = big.tile([P, FR, DIM, N, M16], BF16, tag="ee")
            nc.scalar.activation(out=ee, in_=e_pre,
                                 func=mybir.ActivationFunctionType.Exp)

            # ---- Z = sum_{n,m} E  (bf16 pairwise tree over m, then reduce) ----
            zt1 = work.tile([P, FR, DIM, N, 8], BF16, tag="zt1")
            nc.vector.tensor_tensor(out=zt1, in0=ee[:, :, :, :, 0:8],
                                    in1=ee[:, :, :, :, 8:16], op=OP.add)
            zt2 = work.tile([P, FR, DIM, N, 4], BF16, tag="zt2")
            nc.vector.tensor_tensor(out=zt2, in0=zt1[:, :, :, :, 0:4],
                                    in1=zt1[:, :, :, :, 4:8], op=OP.add)
            zt3 = work.tile([P, FR, DIM, N, 2], BF16, tag="zt3")
            nc.vector.tensor_tensor(out=zt3, in0=zt2[:, :, :, :, 0:2],
                                    in1=zt2[:, :, :, :, 2:4], op=OP.add)
            zz = work.tile([P, FR, DIM], F32, tag="zz")
            nc.vector.reduce_sum(out=zz, in_=zt3, axis=AX.XY)
            rz = work.tile([P, FR, DIM], F32, tag="rz")
            nc.vector.reciprocal(out=rz, in_=zz)

            # ---- weighted v: vz16 = vh*rz (bf16), vh16 = vh (bf16) ----
            vz16 = work.tile([P, FR, DIM, M16], BF16, tag="vz16")
            nc.vector.tensor_tensor(
                out=vz16, in0=qkvh[:, 2 * FR:3 * FR],
                in1=rz[:, :, :, None].broadcast_to((P, FR, DIM, M16)), op=OP.mult)
            vh16 = work.tile([P, FR, DIM, M16], BF16, tag="vh16")
            nc.scalar.copy(vh16, qkvh[:, 2 * FR:3 * FR])
            dist16 = work.tile([P, FR, N, M16], BF16, tag="dist16")
            nc.scalar.mul(dist16[:, :, :, 0:N], dt, 2.0)
            nc.vector.memset(dist16[:, :, :, N:M16], 0.0)

            # ---- P = E*vz16 + dist16*vh16, then pairwise m-tree ----
            p1 = big.tile([P, FR, DIM, N, M16], BF16, tag="p1")
            p2 = big.tile([P, FR, DIM, N, M16], BF16, tag="p2")
            for h in range(DIM):
                nc.vector.tensor_tensor(
                    out=p1[:, :, h], in0=ee[:, :, h],
                    in1=vz16[:, :, h][:, :, None, :].broadcast_to(
                        (P, FR, N, M16)),
                    op=OP.mult)
                nc.vector.tensor_tensor(
                    out=p2[:, :, h],
                    in0=dist16,
                    in1=vh16[:, :, h][:, :, None, :].broadcast_to(
                        (P, FR, N, M16)),
                    op=OP.mult)
            # fold the P1+P2 add into the first tree stage (3 adds of half
            # width instead of full-width add + first stage)
            pa = work.tile([P, FR, DIM, N, 8], BF16, tag="pa")
            nc.vector.tensor_tensor(out=pa, in0=p1[:, :, :, :, 0:8],
                                    in1=p2[:, :, :, :, 0:8], op=OP.add)
            pb = work.tile([P, FR, DIM, N, 8], BF16, tag="pb")
            nc.vector.tensor_tensor(out=pb, in0=p1[:, :, :, :, 8:16],
                                    in1=p2[:, :, :, :, 8:16], op=OP.add)
            pt1 = work.tile([P, FR, DIM, N, 8], BF16, tag="pt1")
            nc.vector.tensor_tensor(out=pt1, in0=pa, in1=pb, op=OP.add)
            pt2 = work.tile([P, FR, DIM, N, 4], BF16, tag="pt2")
            nc.vector.tensor_tensor(out=pt2, in0=pt1[:, :, :, :, 0:4],
                                    in1=pt1[:, :, :, :, 4:8], op=OP.add)
            pt3 = work.tile([P, FR, DIM, N, 2], BF16, tag="pt3")
            nc.vector.tensor_tensor(out=pt3, in0=pt2[:, :, :, :, 0:2],
                                    in1=pt2[:, :, :, :, 2:4], op=OP.add)
            att = work.tile([P, FR, DIM, N], F32, tag="att")
            nc.vector.tensor_tensor(out=att, in0=pt3[:, :, :, :, 0],
                                    in1=pt3[:, :, :, :, 1], op=OP.add)

            # ---- output projection on PE: per 2-frame chunk ----
            att_f = att.rearrange("p f h n -> p (f h n)")       # [P, 180]
            outt = outp.tile([P, FR * FEAT], F32, tag="outt")   # (f, e)
            for c in range(FR // 2):
                at_ps = psum.tile([2 * FEAT, P], F32, tag="at_ps")
                nc.tensor.transpose(out=at_ps, in_=att_f[:, c * 90:(c + 1) * 90],
                                    identity=idn)
                at_sb = outp.tile([2 * FEAT, P], F32, tag="at_sb")
                nc.scalar.copy(at_sb, at_ps)
                ot_ps = psum.tile([2 * FEAT, P], F32, tag="ot_ps")
                nc.tensor.matmul(out=ot_ps, lhsT=mo, rhs=at_sb,
                                 start=True, stop=True)
                ot_sb = outp.tile([2 * FEAT, P], F32, tag="ot_sb")
                nc.scalar.add(ot_sb, ot_ps, bo2)
                ob_ps = psum.tile([P, 2 * FEAT], F32, tag="ob_ps")
                nc.tensor.transpose(out=ob_ps, in_=ot_sb,
                                    identity=idn[0:2 * FEAT, 0:2 * FEAT])
                nc.scalar.copy(outt[:, c * 90:(c + 1) * 90], ob_ps)

            nc.sync.dma_start(out=ov[it], in_=outt)


def _prep_consts(Wq, bq, Wk, bk, Wv, bv, Wo, bo):
    wc = np.stack([Wq, Wk, Wv]).astype(np.float32)                 # [t, h, d]
    wc = np.repeat(wc[:, None, :, :], FR, axis=1).reshape(-1)      # (t,f,h,d)
    bcv = np.stack([bq, bk, bv]).astype(np.float32)                # [t, h]
    bcv = np.repeat(bcv[:, None, :], FR, axis=1).reshape(-1)       # (t,f,h)
    mo1 = np.zeros((FEAT, FEAT), np.float32)                        # [(h,n), e]
    for h in range(DIM):
        for n in range(N):
            mo1[h * N + n, :] = Wo[:, n * DIM + h]
    mo2 = np.zeros((2 * FEAT, 2 * FEAT), np.float32)
    mo2[:FEAT, :FEAT] = mo1
    mo2[FEAT:, FEAT:] = mo1
    bo2 = np.concatenate([bo, bo]).astype(np.float32)
    return wc, bcv, mo2, bo2


_NC_CACHE = {}


def _get_nc():
    if "nc" not in _NC_CACHE:
        _NC_CACHE["nc"] = build_nc()
    return _NC_CACHE["nc"]


def run(inputs, trace=False, tmpdir=None):
    q = np.ascontiguousarray(np.asarray(inputs["q"], np.float32).reshape(-1, FEAT))
    k = np.ascontiguousarray(np.asarray(inputs["k"], np.float32).reshape(-1, FEAT))
    v = np.ascontiguousarray(np.asarray(inputs["v"], np.float32).reshape(-1, FEAT))
    d = np.ascontiguousarray(
        np.asarray(inputs["distances"], np.float32).reshape(-1, N * N))
    wc, bcv, mo2, bo2 = _prep_consts(
        np.asarray(inputs["Wq"]), np.asarray(inputs["bq"]),
        np.asarray(inputs["Wk"]), np.asarray(inputs["bk"]),
        np.asarray(inputs["Wv"]), np.asarray(inputs["bv"]),
        np.asarray(inputs["Wo"]), np.asarray(inputs["bo"]))
    nc = _get_nc()
    in_maps = []
    for i in range(N_CORES):
        sl = slice(i * F_CORE, (i + 1) * F_CORE)
        in_maps.append({
            "q": q[sl], "k": k[sl], "v": v[sl], "d": d[sl],
            "wc": wc, "bc": bcv, "mo": mo2, "bo": bo2,
        })
    res = run_bass_kernel_spmd(nc, in_maps, list(range(N_CORES)), trace=trace,
                               tmpdir=tmpdir)
    out = np.concatenate([res.results[i]["out"] for i in range(N_CORES)], axis=0)
    out = out.reshape(BS, SQ, FEAT)
    att_val = np.zeros((SQ,), dtype=np.float32)
    return (out, att_val), res


def kernel(**inputs):
    (out, att_val), _ = run(inputs, trace=False)
    return out, att_val


# revision 33
# speedup vs baseline: 1.3265x; 1.0022x over previous
"""Trainium2 Bass kernel for nn_MultiHeadAttention_spatial_shared.

Math (per frame f of BS*SQ=131072, all independent):
  qh[h,n] = sum_d Wq[h,d]*q[f, d*15+n] + bq[h]   (same for k, v)
  aq = |qh|, ak = |kh|
  E[h,n,m] = exp(aq[h,n]*ak[h,m])                (flattened softmax numerator)
  Z[h]     = sum_{n,m} E[h,n,m]
  att[h,n] = sum_m E[h,n,m]*vh[h,m]/Z[h] + 2*sum_m dist[n,m]*vh[h,m]
  out[e]   = sum_{h,n} Wo[e, n*3+h]*att[h,n] + bo[e]

Sharding: pure data parallel over frames across 8 cores (16384 frames/core).

On-chip layout: frames on partitions, 4 frames per partition ("FR"), 512
frames per super-tile, 32 super-tiles per core.  The m axis is padded to 16
(pad of ak is -1e30 so exp()=0; pad of v/dist is 0) so the m-reduction can be
done as a bf16 pairwise tree at 2x DVE rate.  The 45x45 output projection and
its transposes run on the TensorEngine.
"""

import numpy as np

import concourse.bass as bass
import concourse.bacc as bacc
import concourse.tile as tile
from concourse import mybir
from concourse.bass_utils import run_bass_kernel_spmd
from concourse.masks import make_identity

F32 = mybir.dt.float32
BF16 = mybir.dt.bfloat16
U32 = mybir.dt.uint32
AX = mybir.AxisListType
OP = mybir.AluOpType

N_CORES = 8
BS, SQ, DIM, N = 32, 4096, 3, 15
FEAT = DIM * N            # 45
M16 = 16                  # m padded to 16
P = 128                   # partitions
FR = 4                    # frames per partition per super-tile
TILE_F = P * FR           # 512 frames per super-tile
F_CORE = BS * SQ // N_CORES  # 16384


def build_nc(n_tiles=F_CORE // TILE_F):
    f_core = n_tiles * TILE_F
    nc = bacc.Bacc("TRN2", target_bir_lowering=False, debug=False)

    qx = nc.dram_tensor("q", [f_core, FEAT], F32, kind="ExternalInput").ap()
    kx = nc.dram_tensor("k", [f_core, FEAT], F32, kind="ExternalInput").ap()
    vx = nc.dram_tensor("v", [f_core, FEAT], F32, kind="ExternalInput").ap()
    dx = nc.dram_tensor("d", [f_core, N * N], F32, kind="ExternalInput").ap()
    wcx = nc.dram_tensor("wc", [3 * FR * DIM * DIM], F32,
                         kind="ExternalInput").ap()
    bcx = nc.dram_tensor("bc", [3 * FR * DIM], F32, kind="ExternalInput").ap()
    mox = nc.dram_tensor("mo", [2 * FEAT, 2 * FEAT], F32, kind="ExternalInput").ap()
    box = nc.dram_tensor("bo", [2 * FEAT], F32, kind="ExternalInput").ap()
    ox = nc.dram_tensor("out", [f_core, FEAT], F32, kind="ExternalOutput").ap()

    # DRAM views: super-tile t, partition p, frame-in-partition f
    qv = qx.rearrange("(t p f) e -> t p (f e)", p=P, f=FR)
    kv = kx.rearrange("(t p f) e -> t p (f e)", p=P, f=FR)
    vv = vx.rearrange("(t p f) e -> t p (f e)", p=P, f=FR)
    dv = dx.rearrange("(t p f) e -> t p (f e)", p=P, f=FR)
    ov = ox.rearrange("(t p f) e -> t p (f e)", p=P, f=FR)

    with tile.TileContext(nc) as tc:
        _kernel_body(tc, n_tiles, qv, kv, vv, dv, ov, wcx, bcx, mox, box)
    nc.compile()
    return nc


def _kernel_body(tc, n_tiles, qv, kv, vv, dv, ov, wcx, bcx, mox, box):
    nc = tc.nc
    from contextlib import ExitStack
    with ExitStack() as ctx:
        consts = ctx.enter_context(tc.tile_pool(name="consts", bufs=1))
        dma_in = ctx.enter_context(tc.tile_pool(name="dma_in", bufs=3))
        work = ctx.enter_context(tc.tile_pool(name="work", bufs=2))
        big = ctx.enter_context(tc.tile_pool(name="big", bufs=2))
        outp = ctx.enter_context(tc.tile_pool(name="outp", bufs=3))
        psum = ctx.enter_context(tc.tile_pool(name="psum", bufs=2, space="PSUM"))

        def pbcast(ap, p=P):
            return bass.AP(tensor=ap.tensor, offset=ap.offset,
                           ap=[[0, p]] + list(ap.ap))

        # ---- constants (once) ----
        # DMA into staging tiles, then copy via DVE so downstream compute ops
        # only ever carry a single sync-wait class (walrus TT encoding allows
        # one wait per instruction).
        wc0 = consts.tile([P, 3 * FR, DIM, DIM], F32)  # ((t,f), h, d)
        nc.sync.dma_start(out=wc0.rearrange("p a b c -> p (a b c)"),
                          in_=pbcast(wcx))
        wc = consts.tile([P, 3 * FR, DIM, DIM], F32)
        nc.vector.tensor_copy(wc, wc0)
        bc0 = consts.tile([P, 3 * FR, DIM], F32)       # ((t,f), h)
        nc.sync.dma_start(out=bc0.rearrange("p a b -> p (a b)"),
                          in_=pbcast(bcx))
        bc = consts.tile([P, 3 * FR, DIM], F32)
        nc.vector.tensor_copy(bc, bc0)
        mo0 = consts.tile([2 * FEAT, 2 * FEAT], F32)   # blockdiag lhsT [(j,h,n),(j,e)]
        nc.sync.dma_start(out=mo0, in_=mox)
        mo = consts.tile([2 * FEAT, 2 * FEAT], F32)
        nc.vector.tensor_copy(mo, mo0)
        bo20 = consts.tile([2 * FEAT, 1], F32)
        nc.sync.dma_start(out=bo20, in_=box.rearrange("(p one) -> p one", one=1))
        bo2 = consts.tile([2 * FEAT, 1], F32)
        nc.vector.tensor_copy(bo2, bo20)
        idn0 = consts.tile([P, P], F32)
        make_identity(nc, idn0)
        idn = consts.tile([P, P], F32)
        nc.vector.tensor_copy(idn, idn0)

        for it in range(n_tiles):
            # ---- DMA in: q,k,v land in one (t,f)-major tile ----
            qkv = dma_in.tile([P, 3 * FR, DIM, N], F32, tag="qkv")
            dt = dma_in.tile([P, FR, N, N], F32, tag="dt")
            nc.sync.dma_start(
                out=qkv[:, 0:FR].rearrange("p f d n -> p (f d n)"), in_=qv[it])
            nc.sync.dma_start(
                out=qkv[:, FR:2 * FR].rearrange("p f d n -> p (f d n)"),
                in_=kv[it])
            nc.sync.dma_start(
                out=qkv[:, 2 * FR:3 * FR].rearrange("p f d n -> p (f d n)"),
                in_=vv[it])
            nc.sync.dma_start(out=dt.rearrange("p f n m -> p (f n m)"), in_=dv[it])

            # ---- projections: qkvh[((t,f), h, m16)] ----
            qkvh = work.tile([P, 3 * FR, DIM, M16], F32, tag="qkvh")
            tmp = work.tile([P, 3 * FR, DIM, DIM, N], F32, tag="ptmp")
            for h in range(DIM):
                nc.vector.tensor_tensor(
                    out=tmp[:, :, h],
                    in0=qkv,
                    in1=wc[:, :, h][:, :, :, None].broadcast_to(
                        (P, 3 * FR, DIM, N)),
                    op=OP.mult)
            # d-contraction as two pairwise adds over all (t,f,h) at once
            t01 = work.tile([P, 3 * FR, DIM, N], F32, tag="t01")
            nc.vector.tensor_tensor(
                out=t01, in0=tmp[:, :, :, 0], in1=tmp[:, :, :, 1], op=OP.add)
            nc.vector.tensor_tensor(
                out=qkvh[:, :, :, 0:N], in0=t01, in1=tmp[:, :, :, 2], op=OP.add)
            # bias
            nc.vector.tensor_tensor(
                out=qkvh[:, :, :, 0:N],
                in0=qkvh[:, :, :, 0:N],
                in1=bc[:, :, :, None].broadcast_to((P, 3 * FR, DIM, N)),
                op=OP.add)
            # abs of qh on ScalarE (15 real columns, in place, f32)
            nc.scalar.activation(
                out=qkvh[:, 0:FR, :, 0:N], in_=qkvh[:, 0:FR, :, 0:N],
                func=mybir.ActivationFunctionType.Abs)
            # ak as bf16 with abs fused into the cast; pad -> -1e30 (exp -> 0)
            ak16 = work.tile([P, FR, DIM, M16], BF16, tag="ak16")
            nc.scalar.activation(
                out=ak16[:, :, :, 0:N], in_=qkvh[:, FR:2 * FR, :, 0:N],
                func=mybir.ActivationFunctionType.Abs)
            nc.vector.memset(ak16[:, :, :, N:M16], -1e30)
            # v pad -> 0 (read by vz16/vh16)
            nc.vector.memset(qkvh[:, 2 * FR:3 * FR, :, N:M16], 0.0)

            # ---- E_pre = aq (x) ak16 in bf16 (2x DVE via materialized
            # broadcast of aq), E = exp(E_pre) ----
            aqb = big.tile([P, FR, DIM, N, M16], BF16, tag="aqb")
            for h in range(DIM):
                nc.scalar.copy(
                    aqb[:, :, h],
                    qkvh[:, 0:FR, h, 0:N][:, :, :, None].broadcast_to(
                        (P, FR, N, M16)))
            e_pre = big.tile([P, FR, DIM, N, M16], BF16, tag="e_pre")
            for h in range(DIM):
                nc.vector.tensor_tensor(
                    out=e_pre[:, :, h],
                    in0=aqb[:, :, h],
                    in1=ak16[:, :, h][:, :, None, :].broadcast_to(
                        (P, FR, N, M16)),
                    op=OP.mult)
            ee = big.tile([P, FR, DIM, N, M16], BF16, tag="ee")
            nc.scalar.activation(out=ee, in_=e_pre,
                                 func=mybir.ActivationFunctionType.Exp)

            # ---- Z = sum_{n,m} E  (bf16 pairwise tree over m, then reduce) ----
            zt1 = work.tile([P, FR, DIM, N, 8], BF16, tag="zt1")
            nc.vector.tensor_tensor(out=zt1, in0=ee[:, :, :, :, 0:8],
                                    in1=ee[:, :, :, :, 8:16], op=OP.add)
            zt2 = work.tile([P, FR, DIM, N, 4], BF16, tag="zt2")
            nc.vector.tensor_tensor(out=zt2, in0=zt1[:, :, :, :, 0:4],
                                    in1=zt1[:, :, :, :, 4:8], op=OP.add)
            zt3 = work.tile([P, FR, DIM, N, 2], BF16, tag="zt3")
            nc.vector.tensor_tensor(out=zt3, in0=zt2[:, :, :, :, 0:2],
                                    in1=zt2[:, :, :, :, 2:4], op=OP.add)
            zz = work.tile([P, FR, DIM], F32, tag="zz")
            nc.vector.reduce_sum(out=zz, in_=zt3, axis=AX.XY)
            rz = work.tile([P, FR, DIM], F32, tag="rz")
            nc.vector.reciprocal(out=rz, in_=zz)

            # ---- weighted v: vz16 = vh*rz (bf16), vh16 = vh (bf16) ----
            vz16 = work.tile([P, FR, DIM, M16], BF16, tag="vz16")
            nc.vector.tensor_tensor(
                out=vz16, in0=qkvh[:, 2 * FR:3 * FR],
                in1=rz[:, :, :, None].broadcast_to((P, FR, DIM, M16)), op=OP.mult)
            # vh broadcast over n, materialized on ScalarE (enables a single
            # full-width 2x P2 multiply)
            vhb = big.tile([P, FR, DIM, N, M16], BF16, tag="vhb")
            for h in range(DIM):
                nc.scalar.copy(
                    vhb[:, :, h],
                    qkvh[:, 2 * FR:3 * FR, h][:, :, None, :].broadcast_to(
                        (P, FR, N, M16)))
            dist16 = work.tile([P, FR, N, M16], BF16, tag="dist16")
            nc.scalar.mul(dist16[:, :, :, 0:N], dt, 2.0)
            nc.vector.memset(dist16[:, :, :, N:M16], 0.0)

            # ---- P = E*vz16 + dist16*vh16, then pairwise m-tree ----
            p1 = big.tile([P, FR, DIM, N, M16], BF16, tag="p1")
            for h in range(DIM):
                nc.vector.tensor_tensor(
                    out=p1[:, :, h], in0=ee[:, :, h],
                    in1=vz16[:, :, h][:, :, None, :].broadcast_to(
                        (P, FR, N, M16)),
                    op=OP.mult)
            p2 = big.tile([P, FR, DIM, N, M16], BF16, tag="p2")
            nc.vector.tensor_tensor(
                out=p2,
                in0=dist16[:, :, None, :, :].broadcast_to(
                    (P, FR, DIM, N, M16)),
                in1=vhb, op=OP.mult)
            # fold the P1+P2 add into the first tree stage (3 adds of half
            # width instead of full-width add + first stage)
            pa = work.tile([P, FR, DIM, N, 8], BF16, tag="pa")
            nc.vector.tensor_tensor(out=pa, in0=p1[:, :, :, :, 0:8],
                                    in1=p2[:, :, :, :, 0:8], op=OP.add)
            pb = work.tile([P, FR, DIM, N, 8], BF16, tag="pb")
            nc.vector.tensor_tensor(out=pb, in0=p1[:, :, :, :, 8:16],
                                    in1=p2[:, :, :, :, 8:16], op=OP.add)
            pt1 = work.tile([P, FR, DIM, N, 8], BF16, tag="pt1")
            nc.vector.tensor_tensor(out=pt1, in0=pa, in1=pb, op=OP.add)
            pt2 = work.tile([P, FR, DIM, N, 4], BF16, tag="pt2")
            nc.vector.tensor_tensor(out=pt2, in0=pt1[:, :, :, :, 0:4],
                                    in1=pt1[:, :, :, :, 4:8], op=OP.add)
            pt3 = work.tile([P, FR, DIM, N, 2], BF16, tag="pt3")
            nc.vector.tensor_tensor(out=pt3, in0=pt2[:, :, :, :, 0:2],
                                    in1=pt2[:, :, :, :, 2:4], op=OP.add)
            att = work.tile([P, FR, DIM, N], F32, tag="att")
            nc.vector.tensor_tensor(out=att, in0=pt3[:, :, :, :, 0],
                                    in1=pt3[:, :, :, :, 1], op=OP.add)

            # ---- output projection on PE: per 2-frame chunk ----
            att_f = att.rearrange("p f h n -> p (f h n)")       # [P, 180]
            outt = outp.tile([P, FR * FEAT], F32, tag="outt")   # (f, e)
            for c in range(FR // 2):
                at_ps = psum.tile([2 * FEAT, P], F32, tag="at_ps")
                nc.tensor.transpose(out=at_ps, in_=att_f[:, c * 90:(c + 1) * 90],
                                    identity=idn)
                at_sb = outp.tile([2 * FEAT, P], F32, tag="at_sb")
                nc.scalar.copy(at_sb, at_ps)
                ot_ps = psum.tile([2 * FEAT, P], F32, tag="ot_ps")
                nc.tensor.matmul(out=ot_ps, lhsT=mo, rhs=at_sb,
                                 start=True, stop=True)
                ot_sb = outp.tile([2 * FEAT, P], F32, tag="ot_sb")
                nc.scalar.add(ot_sb, ot_ps, bo2)
                ob_ps = psum.tile([P, 2 * FEAT], F32, tag="ob_ps")
                nc.tensor.transpose(out=ob_ps, in_=ot_sb,
                                    identity=idn[0:2 * FEAT, 0:2 * FEAT])
                nc.scalar.copy(outt[:, c * 90:(c + 1) * 90], ob_ps)

            nc.sync.dma_start(out=ov[it], in_=outt)


def _prep_consts(Wq, bq, Wk, bk, Wv, bv, Wo, bo):
    wc = np.stack([Wq, Wk, Wv]).astype(np.float32)                 # [t, h, d]
    wc = np.repeat(wc[:, None, :, :], FR, axis=1).reshape(-1)      # (t,f,h,d)
    bcv = np.stack([bq, bk, bv]).astype(np.float32)                # [t, h]
    bcv = np.repeat(bcv[:, None, :], FR, axis=1).reshape(-1)       # (t,f,h)
    mo1 = np.zeros((FEAT, FEAT), np.float32)                        # [(h,n), e]
    for h in range(DIM):
        for n in range(N):
            mo1[h * N + n, :] = Wo[:, n * DIM + h]
    mo2 = np.zeros((2 * FEAT, 2 * FEAT), np.float32)
    mo2[:FEAT, :FEAT] = mo1
    mo2[FEAT:, FEAT:] = mo1
    bo2 = np.concatenate([bo, bo]).astype(np.float32)
    return wc, bcv, mo2, bo2


_NC_CACHE = {}


def _get_nc():
    if "nc" not in _NC_CACHE:
        _NC_CACHE["nc"] = build_nc()
    return _NC_CACHE["nc"]


def run(inputs, trace=False, tmpdir=None):
    q = np.ascontiguousarray(np.asarray(inputs["q"], np.float32).reshape(-1, FEAT))
    k = np.ascontiguousarray(np.asarray(inputs["k"], np.float32).reshape(-1, FEAT))
    v = np.ascontiguousarray(np.asarray(inputs["v"], np.float32).reshape(-1, FEAT))
    d = np.ascontiguousarray(
        np.asarray(inputs["distances"], np.float32).reshape(-1, N * N))
    wc, bcv, mo2, bo2 = _prep_consts(
        np.asarray(inputs["Wq"]), np.asarray(inputs["bq"]),
        np.asarray(inputs["Wk"]), np.asarray(inputs["bk"]),
        np.asarray(inputs["Wv"]), np.asarray(inputs["bv"]),
        np.asarray(inputs["Wo"]), np.asarray(inputs["bo"]))
    nc = _get_nc()
    in_maps = []
    for i in range(N_CORES):
        sl = slice(i * F_CORE, (i + 1) * F_CORE)
        in_maps.append({
            "q": q[sl], "k": k[sl], "v": v[sl], "d": d[sl],
            "wc": wc, "bc": bcv, "mo": mo2, "bo": bo2,
        })
    res = run_bass_kernel_spmd(nc, in_maps, list(range(N_CORES)), trace=trace,
                               tmpdir=tmpdir)
    out = np.concatenate([res.results[i]["out"] for i in range(N_CORES)], axis=0)
    out = out.reshape(BS, SQ, FEAT)
    att_val = np.zeros((SQ,), dtype=np.float32)
    return (out, att_val), res


def kernel(**inputs):
    (out, att_val), _ = run(inputs, trace=False)
    return out, att_val


# revision 35
# speedup vs baseline: 1.3303x; 1.0029x over previous
"""Trainium2 Bass kernel for nn_MultiHeadAttention_spatial_shared.

Math (per frame f of BS*SQ=131072, all independent):
  qh[h,n] = sum_d Wq[h,d]*q[f, d*15+n] + bq[h]   (same for k, v)
  aq = |qh|, ak = |kh|
  E[h,n,m] = exp(aq[h,n]*ak[h,m])                (flattened softmax numerator)
  Z[h]     = sum_{n,m} E[h,n,m]
  att[h,n] = sum_m E[h,n,m]*vh[h,m]/Z[h] + 2*sum_m dist[n,m]*vh[h,m]
  out[e]   = sum_{h,n} Wo[e, n*3+h]*att[h,n] + bo[e]

Sharding: pure data parallel over frames across 8 cores (16384 frames/core).

On-chip layout: frames on partitions, 4 frames per partition ("FR"), 512
frames per super-tile, 32 super-tiles per core.  The m axis is padded to 16
(pad of ak is -1e30 so exp()=0; pad of v/dist is 0) so the m-reduction can be
done as a bf16 pairwise tree at 2x DVE rate.  The 45x45 output projection and
its transposes run on the TensorEngine.
"""

import numpy as np

import concourse.bass as bass
import concourse.bacc as bacc
import concourse.tile as tile
from concourse import mybir
from concourse.bass_utils import run_bass_kernel_spmd
from concourse.masks import make_identity

F32 = mybir.dt.float32
BF16 = mybir.dt.bfloat16
U32 = mybir.dt.uint32
AX = mybir.AxisListType
OP = mybir.AluOpType

N_CORES = 8
BS, SQ, DIM, N = 32, 4096, 3, 15
FEAT = DIM * N            # 45
M16 = 16                  # m padded to 16
P = 128                   # partitions
FR = 4                    # frames per partition per super-tile
TILE_F = P * FR           # 512 frames per super-tile
F_CORE = BS * SQ // N_CORES  # 16384


def build_nc(n_tiles=F_CORE // TILE_F):
    f_core = n_tiles * TILE_F
    nc = bacc.Bacc("TRN2", target_bir_lowering=False, debug=False)

    qx = nc.dram_tensor("q", [f_core, FEAT], F32, kind="ExternalInput").ap()
    kx = nc.dram_tensor("k", [f_core, FEAT], F32, kind="ExternalInput").ap()
    vx = nc.dram_tensor("v", [f_core, FEAT], F32, kind="ExternalInput").ap()
    dx = nc.dram_tensor("d", [f_core, N * N], F32, kind="ExternalInput").ap()
    wcx = nc.dram_tensor("wc", [3 * FR * DIM * DIM], F32,
                         kind="ExternalInput").ap()
    bcx = nc.dram_tensor("bc", [3 * FR * DIM], F32, kind="ExternalInput").ap()
    mox = nc.dram_tensor("mo", [2 * FEAT, 2 * FEAT], F32, kind="ExternalInput").ap()
    box = nc.dram_tensor("bo", [2 * FEAT], F32, kind="ExternalInput").ap()
    ox = nc.dram_tensor("out", [f_core, FEAT], F32, kind="ExternalOutput").ap()

    # DRAM views: super-tile t, partition p, frame-in-partition f
    qv = qx.rearrange("(t p f) e -> t p (f e)", p=P, f=FR)
    kv = kx.rearrange("(t p f) e -> t p (f e)", p=P, f=FR)
    vv = vx.rearrange("(t p f) e -> t p (f e)", p=P, f=FR)
    dv = dx.rearrange("(t p f) e -> t p (f e)", p=P, f=FR)
    ov = ox.rearrange("(t p f) e -> t p (f e)", p=P, f=FR)

    with tile.TileContext(nc) as tc:
        _kernel_body(tc, n_tiles, qv, kv, vv, dv, ov, wcx, bcx, mox, box)
    nc.compile()
    return nc


def _kernel_body(tc, n_tiles, qv, kv, vv, dv, ov, wcx, bcx, mox, box):
    nc = tc.nc
    from contextlib import ExitStack
    with ExitStack() as ctx:
        consts = ctx.enter_context(tc.tile_pool(name="consts", bufs=1))
        dma_in = ctx.enter_context(tc.tile_pool(name="dma_in", bufs=3))
        work = ctx.enter_context(tc.tile_pool(name="work", bufs=3))
        big = ctx.enter_context(tc.tile_pool(name="big", bufs=2))
        outp = ctx.enter_context(tc.tile_pool(name="outp", bufs=3))
        psum = ctx.enter_context(tc.tile_pool(name="psum", bufs=2, space="PSUM"))

        def pbcast(ap, p=P):
            return bass.AP(tensor=ap.tensor, offset=ap.offset,
                           ap=[[0, p]] + list(ap.ap))

        # ---- constants (once) ----
        # DMA into staging tiles, then copy via DVE so downstream compute ops
        # only ever carry a single sync-wait class (walrus TT encoding allows
        # one wait per instruction).
        wc0 = consts.tile([P, 3 * FR, DIM, DIM], F32)  # ((t,f), h, d)
        nc.sync.dma_start(out=wc0.rearrange("p a b c -> p (a b c)"),
                          in_=pbcast(wcx))
        wc = consts.tile([P, 3 * FR, DIM, DIM], F32)
        nc.vector.tensor_copy(wc, wc0)
        bc0 = consts.tile([P, 3 * FR, DIM], F32)       # ((t,f), h)
        nc.sync.dma_start(out=bc0.rearrange("p a b -> p (a b)"),
                          in_=pbcast(bcx))
        bc = consts.tile([P, 3 * FR, DIM], F32)
        nc.vector.tensor_copy(bc, bc0)
        mo0 = consts.tile([2 * FEAT, 2 * FEAT], F32)   # blockdiag lhsT [(j,h,n),(j,e)]
        nc.sync.dma_start(out=mo0, in_=mox)
        mo = consts.tile([2 * FEAT, 2 * FEAT], F32)
        nc.vector.tensor_copy(mo, mo0)
        bo20 = consts.tile([2 * FEAT, 1], F32)
        nc.sync.dma_start(out=bo20, in_=box.rearrange("(p one) -> p one", one=1))
        bo2 = consts.tile([2 * FEAT, 1], F32)
        nc.vector.tensor_copy(bo2, bo20)
        idn0 = consts.tile([P, P], F32)
        make_identity(nc, idn0)
        idn = consts.tile([P, P], F32)
        nc.vector.tensor_copy(idn, idn0)

        for it in range(n_tiles):
            # ---- DMA in: q,k,v land in one (t,f)-major tile ----
            qkv = dma_in.tile([P, 3 * FR, DIM, N], F32, tag="qkv")
            dt = dma_in.tile([P, FR, N, N], F32, tag="dt")
            nc.sync.dma_start(
                out=qkv[:, 0:FR].rearrange("p f d n -> p (f d n)"), in_=qv[it])
            nc.sync.dma_start(
                out=qkv[:, FR:2 * FR].rearrange("p f d n -> p (f d n)"),
                in_=kv[it])
            nc.sync.dma_start(
                out=qkv[:, 2 * FR:3 * FR].rearrange("p f d n -> p (f d n)"),
                in_=vv[it])
            nc.sync.dma_start(out=dt.rearrange("p f n m -> p (f n m)"), in_=dv[it])

            # ---- projections: qkvh[((t,f), h, m16)] ----
            qkvh = work.tile([P, 3 * FR, DIM, M16], F32, tag="qkvh")
            tmp = work.tile([P, 3 * FR, DIM, DIM, N], F32, tag="ptmp")
            for h in range(DIM):
                nc.vector.tensor_tensor(
                    out=tmp[:, :, h],
                    in0=qkv,
                    in1=wc[:, :, h][:, :, :, None].broadcast_to(
                        (P, 3 * FR, DIM, N)),
                    op=OP.mult)
            # d-contraction as two pairwise adds over all (t,f,h) at once
            t01 = work.tile([P, 3 * FR, DIM, N], F32, tag="t01")
            nc.vector.tensor_tensor(
                out=t01, in0=tmp[:, :, :, 0], in1=tmp[:, :, :, 1], op=OP.add)
            nc.vector.tensor_tensor(
                out=qkvh[:, :, :, 0:N], in0=t01, in1=tmp[:, :, :, 2], op=OP.add)
            # bias
            nc.vector.tensor_tensor(
                out=qkvh[:, :, :, 0:N],
                in0=qkvh[:, :, :, 0:N],
                in1=bc[:, :, :, None].broadcast_to((P, 3 * FR, DIM, N)),
                op=OP.add)
            # abs of qh on ScalarE (15 real columns, in place, f32)
            nc.scalar.activation(
                out=qkvh[:, 0:FR, :, 0:N], in_=qkvh[:, 0:FR, :, 0:N],
                func=mybir.ActivationFunctionType.Abs)
            # ak as bf16 with abs fused into the cast; pad -> -1e30 (exp -> 0)
            ak16 = work.tile([P, FR, DIM, M16], BF16, tag="ak16")
            nc.scalar.activation(
                out=ak16[:, :, :, 0:N], in_=qkvh[:, FR:2 * FR, :, 0:N],
                func=mybir.ActivationFunctionType.Abs)
            nc.vector.memset(ak16[:, :, :, N:M16], -1e30)
            # v pad -> 0 (read by vz16/vh16)
            nc.vector.memset(qkvh[:, 2 * FR:3 * FR, :, N:M16], 0.0)

            # ---- E_pre = aq (x) ak16 in bf16 (2x DVE via materialized
            # broadcast of aq), E = exp(E_pre) ----
            aqb = big.tile([P, FR, DIM, N, M16], BF16, tag="aqb")
            for h in range(DIM):
                nc.scalar.copy(
                    aqb[:, :, h],
                    qkvh[:, 0:FR, h, 0:N][:, :, :, None].broadcast_to(
                        (P, FR, N, M16)))
            e_pre = big.tile([P, FR, DIM, N, M16], BF16, tag="e_pre")
            for h in range(DIM):
                nc.vector.tensor_tensor(
                    out=e_pre[:, :, h],
                    in0=aqb[:, :, h],
                    in1=ak16[:, :, h][:, :, None, :].broadcast_to(
                        (P, FR, N, M16)),
                    op=OP.mult)
            ee = big.tile([P, FR, DIM, N, M16], BF16, tag="ee")
            nc.scalar.activation(out=ee, in_=e_pre,
                                 func=mybir.ActivationFunctionType.Exp)

            # ---- Z = sum_{n,m} E  (bf16 pairwise tree over m, then reduce) ----
            zt1 = work.tile([P, FR, DIM, N, 8], BF16, tag="zt1")
            nc.vector.tensor_tensor(out=zt1, in0=ee[:, :, :, :, 0:8],
                                    in1=ee[:, :, :, :, 8:16], op=OP.add)
            zt2 = work.tile([P, FR, DIM, N, 4], BF16, tag="zt2")
            nc.vector.tensor_tensor(out=zt2, in0=zt1[:, :, :, :, 0:4],
                                    in1=zt1[:, :, :, :, 4:8], op=OP.add)
            zt3 = work.tile([P, FR, DIM, N, 2], BF16, tag="zt3")
            nc.vector.tensor_tensor(out=zt3, in0=zt2[:, :, :, :, 0:2],
                                    in1=zt2[:, :, :, :, 2:4], op=OP.add)
            zz = work.tile([P, FR, DIM], F32, tag="zz")
            nc.vector.reduce_sum(out=zz, in_=zt3, axis=AX.XY)
            rz = work.tile([P, FR, DIM], F32, tag="rz")
            nc.vector.reciprocal(out=rz, in_=zz)

            # ---- weighted v: vz16 = vh*rz (bf16), vh16 = vh (bf16) ----
            vz16 = work.tile([P, FR, DIM, M16], BF16, tag="vz16")
            nc.vector.tensor_tensor(
                out=vz16, in0=qkvh[:, 2 * FR:3 * FR],
                in1=rz[:, :, :, None].broadcast_to((P, FR, DIM, M16)), op=OP.mult)
            # vh broadcast over n, materialized on ScalarE (enables a single
            # full-width 2x P2 multiply)
            vhb = big.tile([P, FR, DIM, N, M16], BF16, tag="vhb")
            for h in range(DIM):
                nc.scalar.copy(
                    vhb[:, :, h],
                    qkvh[:, 2 * FR:3 * FR, h][:, :, None, :].broadcast_to(
                        (P, FR, N, M16)))
            dist16 = work.tile([P, FR, N, M16], BF16, tag="dist16")
            nc.scalar.mul(dist16[:, :, :, 0:N], dt, 2.0)
            nc.vector.memset(dist16[:, :, :, N:M16], 0.0)

            # ---- P = E*vz16 + dist16*vh16, then pairwise m-tree ----
            p1 = big.tile([P, FR, DIM, N, M16], BF16, tag="p1")
            for h in range(DIM):
                nc.vector.tensor_tensor(
                    out=p1[:, :, h], in0=ee[:, :, h],
                    in1=vz16[:, :, h][:, :, None, :].broadcast_to(
                        (P, FR, N, M16)),
                    op=OP.mult)
            p2 = big.tile([P, FR, DIM, N, M16], BF16, tag="p2")
            nc.vector.tensor_tensor(
                out=p2,
                in0=dist16[:, :, None, :, :].broadcast_to(
                    (P, FR, DIM, N, M16)),
                in1=vhb, op=OP.mult)
            # fold the P1+P2 add into the first tree stage (3 adds of half
            # width instead of full-width add + first stage)
            pa = work.tile([P, FR, DIM, N, 8], BF16, tag="pa")
            nc.vector.tensor_tensor(out=pa, in0=p1[:, :, :, :, 0:8],
                                    in1=p2[:, :, :, :, 0:8], op=OP.add)
            pb = work.tile([P, FR, DIM, N, 8], BF16, tag="pb")
            nc.vector.tensor_tensor(out=pb, in0=p1[:, :, :, :, 8:16],
                                    in1=p2[:, :, :, :, 8:16], op=OP.add)
            pt1 = work.tile([P, FR, DIM, N, 8], BF16, tag="pt1")
            nc.vector.tensor_tensor(out=pt1, in0=pa, in1=pb, op=OP.add)
            pt2 = work.tile([P, FR, DIM, N, 4], BF16, tag="pt2")
            nc.vector.tensor_tensor(out=pt2, in0=pt1[:, :, :, :, 0:4],
                                    in1=pt1[:, :, :, :, 4:8], op=OP.add)
            pt3 = work.tile([P, FR, DIM, N, 2], BF16, tag="pt3")
            nc.vector.tensor_tensor(out=pt3, in0=pt2[:, :, :, :, 0:2],
                                    in1=pt2[:, :, :, :, 2:4], op=OP.add)
            att = work.tile([P, FR, DIM, N], F32, tag="att")
            nc.vector.tensor_tensor(out=att, in0=pt3[:, :, :, :, 0],
                                    in1=pt3[:, :, :, :, 1], op=OP.add)

            # ---- output projection on PE: per 2-frame chunk ----
            att_f = att.rearrange("p f h n -> p (f h n)")       # [P, 180]
            outt = outp.tile([P, FR * FEAT], F32, tag="outt")   # (f, e)
            for c in range(FR // 2):
                at_ps = psum.tile([2 * FEAT, P], F32, tag="at_ps")
                nc.tensor.transpose(out=at_ps, in_=att_f[:, c * 90:(c + 1) * 90],
                                    identity=idn)
                at_sb = outp.tile([2 * FEAT, P], F32, tag="at_sb")
                nc.scalar.copy(at_sb, at_ps)
                ot_ps = psum.tile([2 * FEAT, P], F32, tag="ot_ps")
                nc.tensor.matmul(out=ot_ps, lhsT=mo, rhs=at_sb,
                                 start=True, stop=True)
                ot_sb = outp.tile([2 * FEAT, P], F32, tag="ot_sb")
                nc.scalar.add(ot_sb, ot_ps, bo2)
                ob_ps = psum.tile([P, 2 * FEAT], F32, tag="ob_ps")
                nc.tensor.transpose(out=ob_ps, in_=ot_sb,
                                    identity=idn[0:2 * FEAT, 0:2 * FEAT])
                nc.scalar.copy(outt[:, c * 90:(c + 1) * 90], ob_ps)

            nc.sync.dma_start(out=ov[it], in_=outt)


def _prep_consts(Wq, bq, Wk, bk, Wv, bv, Wo, bo):
    wc = np.stack([Wq, Wk, Wv]).astype(np.float32)                 # [t, h, d]
    wc = np.repeat(wc[:, None, :, :], FR, axis=1).reshape(-1)      # (t,f,h,d)
    bcv = np.stack([bq, bk, bv]).astype(np.float32)                # [t, h]
    bcv = np.repeat(bcv[:, None, :], FR, axis=1).reshape(-1)       # (t,f,h)
    mo1 = np.zeros((FEAT, FEAT), np.float32)                        # [(h,n), e]
    for h in range(DIM):
        for n in range(N):
            mo1[h * N + n, :] = Wo[:, n * DIM + h]
    mo2 = np.zeros((2 * FEAT, 2 * FEAT), np.float32)
    mo2[:FEAT, :FEAT] = mo1
    mo2[FEAT:, FEAT:] = mo1
    bo2 = np.concatenate([bo, bo]).astype(np.float32)
    return wc, bcv, mo2, bo2


_NC_CACHE = {}


def _get_nc():
    if "nc" not in _NC_CACHE:
        _NC_CACHE["nc"] = build_nc()
    return _NC_CACHE["nc"]


def run(inputs, trace=False, tmpdir=None):
    q = np.ascontiguousarray(np.asarray(inputs["q"], np.float32).reshape(-1, FEAT))
    k = np.ascontiguousarray(np.asarray(inputs["k"], np.float32).reshape(-1, FEAT))
    v = np.ascontiguousarray(np.asarray(inputs["v"], np.float32).reshape(-1, FEAT))
    d = np.ascontiguousarray(
        np.asarray(inputs["distances"], np.float32).reshape(-1, N * N))
    wc, bcv, mo2, bo2 = _prep_consts(
        np.asarray(inputs["Wq"]), np.asarray(inputs["bq"]),
        np.asarray(inputs["Wk"]), np.asarray(inputs["bk"]),
        np.asarray(inputs["Wv"]), np.asarray(inputs["bv"]),
        np.asarray(inputs["Wo"]), np.asarray(inputs["bo"]))
    nc = _get_nc()
    in_maps = []
    for i in range(N_CORES):
        sl = slice(i * F_CORE, (i + 1) * F_CORE)
        in_maps.append({
            "q": q[sl], "k": k[sl], "v": v[sl], "d": d[sl],
            "wc": wc, "bc": bcv, "mo": mo2, "bo": bo2,
        })
    res = run_bass_kernel_spmd(nc, in_maps, list(range(N_CORES)), trace=trace,
                               tmpdir=tmpdir)
    out = np.concatenate([res.results[i]["out"] for i in range(N_CORES)], axis=0)
    out = out.reshape(BS, SQ, FEAT)
    att_val = np.zeros((SQ,), dtype=np.float32)
    return (out, att_val), res


def kernel(**inputs):
    (out, att_val), _ = run(inputs, trace=False)
    return out, att_val
